# revision 72
# baseline (speedup 1.0000x reference)
"""GCF (graph collaborative filtering) message passing on 8 Trainium2 cores.

Sharding: nodes split contiguously for layers 1-2 (core c owns rows
[c*12500, (c+1)*12500)); layer 3 computes only the rows referenced by the
final batch gather (~27k of 100k), re-sharded evenly across cores.

Per layer SpMM: edges bucketed by dest 256-row block; per 128-edge chunk a
one-hot O[e, row] = vals[e]*(rowl[e]==row) is built in one vector
tensor_scalar, and PE accumulates Lx^T += Xg^T @ O per block in PSUM.
  - Layer 1 sources come from the input table, so the host pre-gathers
    feat0[cols] into a partition-major bf16 stream loaded with plain
    dma_start (no gpsimd descriptor generation); chunks are block-major and
    the dense pass-1 for a block runs as soon as its last chunk lands.
  - Layers 2-3 dma_gather from the AllGather table (4 int16 ranges of 32768
    rows); Lx spills to an SBUF accumulator across ranges.
Dense (two passes, transposed layout): pass 1 per block computes
y^T = Wlin^T(Lx+F)^T + Wint^T(Lx*F)^T in PSUM, fused bias+leaky-relu into
the feature buffer, and a ones-matmul accumulates per-row square-norms into
one [NB,256] PSUM tile. Pass 2 does a single sqrt + reciprocal for all
blocks, then per block a K=1 broadcast matmul + multiply normalizes, and PE
transposes write the normal-layout shard for the AllGather.

Final: batch split 2048/core; per concat level dma_gather u/i rows,
multiply+reduce into an accumulator; levels are emitted as soon as their
table is available so the gathers overlap later sweeps. Host inverts the
slot permutation. Level 3 reads the compact pruned table (single range).

All feature data is bf16 (PSUM accumulation fp32); norms fp32.
"""

import os

import ml_dtypes
import numpy as np

import concourse.bacc as bacc
import concourse.mybir as mybir
import concourse.tile as tile
from concourse.bass import ts
from concourse.bass_utils import run_bass_kernel_spmd
from concourse.masks import make_identity

BF = ml_dtypes.bfloat16

NUM_USERS = 30000
NUM_ITEMS = 70000
N = 100000
D = 128
NL = 3
BATCH = 16384
NCORE = 8
SHARD = N // NCORE            # 12500
RB = 256                      # dest rows per block
NB = (SHARD + RB - 1) // RB   # 49 blocks/core (layers 1-2)
RS = 32768                    # range size (int16 index window)
NR = 4                        # ranges
GMAX = 24                     # max chunks per dma_gather call
SGC = 16                      # chunks per layer-1 stream tile
SGD = 8                       # chunks per layer-1 stream dma_start
BSH = BATCH // NCORE          # 2048
EPS2 = 1e-24
SLOPE = 0.01
SP = os.environ.get("KSP", "0") == "1"

f32 = mybir.dt.float32
f32r = mybir.dt.float32r
bf16 = mybir.dt.bfloat16
i16 = mybir.dt.int16

_cache = {}


def _key(meta):
    return repr(meta)


def _build(meta):
    (NCH0, chunk0, NCH1, groups1, chunk1, first_r1, last_r1,
     NCH2, groups2, chunk2, first_r2, last_r2, NS3, NB3, NF3T, F3R,
     fin_bounds, NFB) = meta
    nc = bacc.Bacc(num_devices=NCORE, num_swdge_queues=4)

    feat0 = nc.dram_tensor("feat0", [N, D], bf16, kind="ExternalInput")
    f0t = nc.dram_tensor("f0t", [D, SHARD], bf16, kind="ExternalInput")
    g0 = nc.dram_tensor("g0", [128, NCH0 * 128], bf16, kind="ExternalInput")
    erow0 = nc.dram_tensor("erow0", [128, NCH0], f32, kind="ExternalInput")
    evals0 = nc.dram_tensor("evals0", [128, NCH0], f32, kind="ExternalInput")
    eidx1 = nc.dram_tensor("eidx1", [128, NCH1 * 8], i16, kind="ExternalInput")
    erow1 = nc.dram_tensor("erow1", [128, NCH1], f32, kind="ExternalInput")
    evals1 = nc.dram_tensor("evals1", [128, NCH1], f32, kind="ExternalInput")
    eidx2 = nc.dram_tensor("eidx2", [128, NCH2 * 8], i16, kind="ExternalInput")
    erow2 = nc.dram_tensor("erow2", [128, NCH2], f32, kind="ExternalInput")
    evals2 = nc.dram_tensor("evals2", [128, NCH2], f32, kind="ExternalInput")
    idxf3 = nc.dram_tensor("idxf3", [128, NF3T // 16], i16,
                           kind="ExternalInput")
    wlin = nc.dram_tensor("wlin", [D, NL * D], bf16, kind="ExternalInput")
    wint = nc.dram_tensor("wint", [D, NL * D], bf16, kind="ExternalInput")
    biasr = nc.dram_tensor("biasr", [1, NL * D], bf16, kind="ExternalInput")
    uidx = nc.dram_tensor("uidx", [128, NFB * 8], i16, kind="ExternalInput")
    iidx = nc.dram_tensor("iidx", [128, NFB * 8], i16, kind="ExternalInput")
    uidx3 = nc.dram_tensor("uidx3", [128, NFB * 8], i16, kind="ExternalInput")
    iidx3 = nc.dram_tensor("iidx3", [128, NFB * 8], i16, kind="ExternalInput")
    score = nc.dram_tensor("score", [128, NFB], f32, kind="ExternalOutput")

    add = mybir.AluOpType.add
    mult = mybir.AluOpType.mult
    maxop = mybir.AluOpType.max
    is_equal = mybir.AluOpType.is_equal
    AF = mybir.ActivationFunctionType

    with tile.TileContext(nc) as tc:
        with (
            tc.tile_pool(name="const", bufs=1) as cp,
            tc.tile_pool(name="ft", bufs=1) as ftp,
            tc.tile_pool(name="st", bufs=3) as stp,
            tc.tile_pool(name="g", bufs=3) as gp,
            tc.tile_pool(name="fin", bufs=1) as fp_,
            tc.tile_pool(name="ot", bufs=8) as otp,
            tc.tile_pool(name="sb", bufs=3) as sbp,
            tc.tile_pool(name="plx", bufs=3, space="PSUM") as plx,
            tc.tile_pool(name="py", bufs=3, space="PSUM") as pyp,
            tc.tile_pool(name="ptp", bufs=2, space="PSUM") as ptp,
            tc.tile_pool(name="dram", bufs=1, space="DRAM") as dp,
        ):
            # ---------- constants ----------
            iota = cp.tile([128, RB], bf16)
            nc.gpsimd.iota(iota[:], pattern=[[1, RB]], base=0,
                           channel_multiplier=0,
                           allow_small_or_imprecise_dtypes=True)
            ident = cp.tile([128, 128], bf16)
            make_identity(nc, ident[:])
            epsb = cp.tile([128, 1], f32)
            nc.vector.memset(epsb[:], EPS2)
            ones_row = cp.tile([1, RB], bf16)
            nc.vector.memset(ones_row[:], 1.0)

            wlin_b = cp.tile([128, NL * 128], bf16)
            nc.sync.dma_start(out=wlin_b[:], in_=wlin[:])
            wint_b = cp.tile([128, NL * 128], bf16)
            nc.sync.dma_start(out=wint_b[:], in_=wint[:])
            biasr_b = cp.tile([1, NL * 128], bf16)
            nc.sync.dma_start(out=biasr_b[:], in_=biasr[:])

            erow0_sb = cp.tile([128, NCH0], f32)
            nc.sync.dma_start(out=erow0_sb[:], in_=erow0[:])
            evals0_sb = cp.tile([128, NCH0], f32)
            nc.sync.dma_start(out=evals0_sb[:], in_=evals0[:])
            eidx1_sb = cp.tile([128, NCH1 * 8], i16)
            nc.sync.dma_start(out=eidx1_sb[:], in_=eidx1[:])
            erow1_sb = cp.tile([128, NCH1], f32)
            nc.sync.dma_start(out=erow1_sb[:], in_=erow1[:])
            evals1_sb = cp.tile([128, NCH1], f32)
            nc.sync.dma_start(out=evals1_sb[:], in_=evals1[:])
            eidx2_sb = cp.tile([128, NCH2 * 8], i16)
            nc.sync.dma_start(out=eidx2_sb[:], in_=eidx2[:])
            erow2_sb = cp.tile([128, NCH2], f32)
            nc.sync.dma_start(out=erow2_sb[:], in_=erow2[:])
            evals2_sb = cp.tile([128, NCH2], f32)
            nc.sync.dma_start(out=evals2_sb[:], in_=evals2[:])
            idxf3_sb = cp.tile([128, NF3T // 16], i16)
            nc.sync.dma_start(out=idxf3_sb[:], in_=idxf3[:])
            uidx_sb = cp.tile([128, NFB * 8], i16)
            nc.sync.dma_start(out=uidx_sb[:], in_=uidx[:])
            iidx_sb = cp.tile([128, NFB * 8], i16)
            nc.sync.dma_start(out=iidx_sb[:], in_=iidx[:])
            uidx3_sb = cp.tile([128, NFB * 8], i16)
            nc.sync.dma_start(out=uidx3_sb[:], in_=uidx3[:])
            iidx3_sb = cp.tile([128, NFB * 8], i16)
            nc.sync.dma_start(out=iidx3_sb[:], in_=iidx3[:])

            fta = ftp.tile([128, NB * RB], bf16, tag="fta")
            ftb = ftp.tile([128, NB * RB], bf16, tag="ftb")
            ft3in = ftp.tile([128, NB3 * RB], bf16, tag="ft3in")
            ft3out = ftp.tile([128, NB3 * RB], bf16, tag="ft3out")
            nc.vector.memset(fta[:, SHARD:], 0.0)
            nc.vector.memset(ft3out[:], 0.0)
            nc.vector.memset(ft3in[:], 0.0)
            nc.sync.dma_start(out=fta[:, :SHARD], in_=f0t[:])

            fshard = dp.tile([NB * RB, D], bf16)
            fshard3 = dp.tile([NS3, D], bf16)
            ags = [dp.tile([N, D], bf16, name=f"ag{i}", tag=f"ag{i}",
                           addr_space="Shared") for i in range(2)]
            ag3 = dp.tile([NCORE * NS3, D], bf16, name="ag3", tag="ag3",
                          addr_space="Shared")

            acc = cp.tile([128, NFB], f32)

            # ---------- helpers ----------
            def onehot(q, qrow, qval, tag):
                o = otp.tile([128, RB], bf16, tag="o", name=f"o_{tag}_{q}")
                nc.vector.tensor_scalar(
                    out=o[:], in0=iota[:],
                    scalar1=qrow[:, q:q + 1],
                    scalar2=qval[:, q:q + 1],
                    op0=is_equal, op1=mult,
                )
                return o

            def pass1(l, b, ftin, ftout):
                lxs = ftout[:, ts(b, RB)]
                fin_ = ftin[:, ts(b, RB)]
                pre1 = sbp.tile([128, RB], bf16, tag="pre1")
                nc.vector.tensor_tensor(out=pre1[:], in0=lxs, in1=fin_,
                                        op=add)
                pre2 = sbp.tile([128, RB], bf16, tag="pre2")
                nc.vector.tensor_tensor(out=pre2[:], in0=lxs, in1=fin_,
                                        op=mult)
                y = pyp.tile([128, RB], f32, tag="y")
                nc.tensor.matmul(out=y[:], lhsT=wlin_b[:, ts(l, 128)],
                                 rhs=pre1[:], start=True, stop=False)
                nc.tensor.matmul(out=y[:], lhsT=wint_b[:, ts(l, 128)],
                                 rhs=pre2[:], start=False, stop=False)
                nc.tensor.matmul(out=y[:], lhsT=biasr_b[:, ts(l, 128)],
                                 rhs=ones_row[:], start=False, stop=True)
                # leaky-relu on DVE: max(y, 0.01*y)
                t1 = sbp.tile([128, RB], bf16, tag="t1")
                nc.vector.tensor_scalar(out=t1[:], in0=y[:], scalar1=SLOPE,
                                        scalar2=None, op0=mult)
                nc.vector.tensor_tensor(out=lxs, in0=y[:], in1=t1[:],
                                        op=maxop)

            def pass2b(l, b, nrows, ftout, fsh):
                # normalize in normal layout: transpose, per-row norm,
                # fused scale into the shard-write copy
                for h in range(2):
                    r0 = b * RB + h * 128
                    nr = min(128, nrows - r0)
                    if nr <= 0:
                        break
                    tp = ptp.tile([128, 128], bf16, tag="tp",
                                  name=f"tp{l}_{b}_{h}")
                    nc.tensor.transpose(
                        out=tp[:], in_=ftout[:, r0:r0 + 128],
                        identity=ident[:])
                    tpc = sbp.tile([128, 128], bf16, tag="tpc")
                    nc.vector.tensor_copy(tpc[:], tp[:])
                    sq2 = sbp.tile([128, 128], f32, tag="sq2")
                    nc.vector.tensor_tensor(out=sq2[:], in0=tpc[:],
                                            in1=tpc[:], op=mult)
                    ssum = sbp.tile([128, 1], f32, tag="ssum")
                    nc.vector.tensor_reduce(
                        out=ssum[:], in_=sq2[:],
                        axis=mybir.AxisListType.X, op=add)
                    rt2 = sbp.tile([128, 1], f32, tag="rt2")
                    nc.scalar.activation(out=rt2[:], in_=ssum[:],
                                         func=AF.Sqrt, bias=epsb[:])
                    inv1 = sbp.tile([128, 1], f32, tag="inv1")
                    nc.vector.reciprocal(inv1[:], rt2[:])
                    cpo = sbp.tile([128, 128], bf16, tag="cpo")
                    nc.vector.tensor_scalar(
                        out=cpo[:], in0=tpc[:], scalar1=inv1[:],
                        scalar2=None, op0=mult)
                    nc.sync.dma_start(out=fsh[r0:r0 + nr, :],
                                      in_=cpo[:nr, :])

            def final_level(li, srcf, u_sb, i_sb, bounds):
                ug = fp_.tile([128, NFB * 128], bf16, tag="ug",
                              name=f"ug{li}")
                nc.gpsimd.dma_gather(
                    ug[:].rearrange("p (c d) -> p c d", d=128),
                    srcf[:],
                    u_sb[:],
                    NFB * 128, NFB * 128, 128,
                    single_packet=SP,
                    queue_num=3,
                )
                ig = fp_.tile([128, NFB * 128], bf16, tag="ig",
                              name=f"ig{li}")
                for qi, (rr, c0, cn) in enumerate(bounds):
                    nc.gpsimd.dma_gather(
                        ig[:, c0 * 128:(c0 + cn) * 128].rearrange(
                            "p (c d) -> p c d", d=128),
                        srcf[rr * RS:, :],
                        i_sb[:, c0 * 8:(c0 + cn) * 8],
                        cn * 128, cn * 128, 128,
                        single_packet=SP,
                        queue_num=qi % 4,
                    )
                nc.vector.tensor_tensor(out=ug[:], in0=ug[:], in1=ig[:],
                                        op=mult)
                sc = sbp.tile([128, NFB], f32, tag="sc")
                nc.vector.tensor_reduce(
                    out=sc[:],
                    in_=ug[:].rearrange("p (c d) -> p c d", d=128),
                    axis=mybir.AxisListType.X, op=add)
                if li == 0:
                    nc.vector.tensor_copy(acc[:], sc[:])
                else:
                    nc.vector.tensor_tensor(out=acc[:], in0=acc[:],
                                            in1=sc[:], op=add)

            # ---------- level-0 final gather (overlaps layer-1 sweep) ----
            final_level(0, feat0, uidx_sb, iidx_sb, fin_bounds)

            # ---------- layer 1: streamed sources, block-major ----------
            cur_lx = None
            nst = (NCH0 + SGC - 1) // SGC
            for si in range(nst):
                q0 = si * SGC
                gn = min(SGC, NCH0 - q0)
                st = stp.tile([128, SGC * 128], bf16, tag="st",
                              name=f"st{si}")
                for j in range(0, gn, SGD):
                    jn = min(SGD, gn - j)
                    nc.sync.dma_start(
                        out=st[:, j * 128:(j + jn) * 128],
                        in_=g0[:, (q0 + j) * 128:(q0 + j + jn) * 128])
                for q in range(q0, q0 + gn):
                    b_q, is_first, is_last = chunk0[q]
                    o = onehot(q, erow0_sb, evals0_sb, "l0")
                    if is_first:
                        cur_lx = plx.tile([128, RB], f32, tag="lx",
                                          name=f"lx0_{q}")
                    nc.tensor.matmul(
                        out=cur_lx[:],
                        lhsT=st[:, ts(q - q0, 128)],
                        rhs=o[:],
                        start=is_first, stop=is_last,
                    )
                    if is_last:
                        nc.vector.tensor_copy(ftb[:, ts(b_q, RB)],
                                              cur_lx[:])
                        pass1(0, b_q, fta, ftb)
                        pass2b(0, b_q, NB * RB, ftb, fshard)
            # reload normalized features, transposed, as layer-2 dense input
            nc.sync.dma_start_transpose(out=fta[:], in_=fshard[:])
            nc.gpsimd.collective_compute(
                "AllGather", mybir.AluOpType.bypass,
                replica_groups=[list(range(NCORE))],
                ins=[fshard[:SHARD, :].opt()], outs=[ags[0].opt()],
            )

            # ---------- layers 2-3: gathered sources, range-major --------
            for l, (NCHl, groups, chunk_info, first_r, last_r, eidx_sb,
                    erow_sb, evals_sb, src, ftin, ftout, nb, nrows, fsh,
                    agout) in enumerate((
                (NCH1, groups1, chunk1, first_r1, last_r1, eidx1_sb,
                 erow1_sb, evals1_sb, ags[0], fta, ftb, NB, NB * RB,
                 fshard, ags[1]),
                (NCH2, groups2, chunk2, first_r2, last_r2, eidx2_sb,
                 erow2_sb, evals2_sb, ags[1], ft3in, ft3out, NB3, NS3,
                 fshard3, ag3),
            ), start=1):
                cur_lx = None
                for gi, (gr, q0, gn) in enumerate(groups):
                    gt = gp.tile([128, GMAX * 128], bf16, tag="g",
                                 name=f"g{l}_{gi}")
                    nc.gpsimd.dma_gather(
                        gt[:, :gn * 128].rearrange("p (c d) -> p c d",
                                                   d=128),
                        src[gr * RS:, :],
                        eidx_sb[:, q0 * 8:(q0 + gn) * 8],
                        gn * 128, gn * 128, 128,
                        single_packet=SP,
                        queue_num=gi % 4,
                    )
                    for q in range(q0, q0 + gn):
                        r_q, b_q, is_first, is_last, loc = chunk_info[q]
                        o = onehot(q, erow_sb, evals_sb, f"l{l}")
                        if is_first:
                            cur_lx = plx.tile([128, RB], f32, tag="lx",
                                              name=f"lx{l}_{q}")
                        nc.tensor.matmul(
                            out=cur_lx[:],
                            lhsT=gt[:, ts(loc, 128)],
                            rhs=o[:],
                            start=is_first, stop=is_last,
                        )
                        if is_last:
                            dst = ftout[:, ts(b_q, RB)]
                            if first_r[b_q] == r_q:
                                nc.scalar.copy(dst, cur_lx[:])
                            else:
                                nc.vector.tensor_tensor(
                                    out=dst, in0=dst, in1=cur_lx[:],
                                    op=add)
                            if last_r[b_q] == r_q:
                                pass1(l, b_q, ftin, ftout)
                                pass2b(l, b_q, nrows, ftout, fsh)
                # final-level gathers fill the gpsimd window while the last
                # blocks' dense chains drain (tables from previous CC)
                final_level(l, ags[l - 1], uidx_sb, iidx_sb, fin_bounds)
                nc.gpsimd.collective_compute(
                    "AllGather", mybir.AluOpType.bypass,
                    replica_groups=[list(range(NCORE))],
                    ins=[fsh[:SHARD, :].opt() if l == 1 else fsh.opt()],
                    outs=[agout.opt()],
                )
                if l == 1:
                    # layer-3 dense inputs: transposed gather of the compact
                    # rows' layer-2 features (per int16 source range)
                    rb0 = 0
                    for r in range(NR):
                        fn = F3R[r]
                        if fn == 0:
                            continue
                        nc.gpsimd.dma_gather(
                            ft3in[:, rb0:rb0 + fn].rearrange(
                                "p (c d) -> p c d", c=1),
                            ags[1][r * RS:, :],
                            idxf3_sb[:, rb0 // 16:(rb0 + fn) // 16],
                            fn, fn, 128,
                            transpose=True,
                            single_packet=SP,
                            queue_num=r % 4,
                        )
                        rb0 += fn
                else:
                    final_level(3, ag3, uidx3_sb, iidx3_sb,
                                [(0, 0, NFB)])
            nc.sync.dma_start(out=score[:], in_=acc[:])

    nc.compile()
    return nc


def _chunks_of(counts, kk):
    """Block-major chunk table: [(b, is_first, is_last)] and base offsets."""
    info = []
    base = {}
    q = 0
    for b, k in enumerate(kk):
        if k == 0:
            continue
        base[b] = q
        for j in range(k):
            info.append((b, j == 0, j == k - 1))
        q += k
    return info, base, q


def _pack_spmm_ranged(rows_l, cols_l, vals_l, nshard, nblocks):
    """Bucket edges by (core, range, block) with shared chunk counts.

    rows_l are destination indices in the (possibly compact) shard space
    (core = row // nshard); cols_l index the source table (range split).
    Returns per-core eidx/erow/evals arrays plus chunk/group metadata.
    """
    core = rows_l // nshard
    local = rows_l - core * nshard
    blk = local // RB
    rowl = (local - blk * RB).astype(np.float32)
    rng = cols_l // RS
    col_local = (cols_l - rng * RS).astype(np.int16)

    nbk = nblocks
    bkey = ((core * NR + rng) * nbk + blk).astype(np.int64)
    order = np.argsort(bkey, kind="stable")
    bkey_s = bkey[order]
    counts = np.bincount(bkey_s, minlength=NCORE * NR * nbk)
    counts = counts.reshape(NCORE, NR, nbk)
    k2 = np.ceil(counts.max(axis=0) / 128).astype(np.int64)  # [NR, nbk]

    chunk_base = np.zeros((NR, nbk), dtype=np.int64)
    nch = 0
    chunk_info = []          # (r, b, is_first, is_last, loc)
    groups = []              # (r, q0, n)
    first_r = [None] * nbk
    last_r = [None] * nbk
    for r in range(NR):
        for b in range(nbk):
            k = int(k2[r, b])
            if k == 0:
                continue
            if first_r[b] is None:
                first_r[b] = r
            last_r[b] = r
            chunk_base[r, b] = nch
            for j in range(k):
                chunk_info.append([r, b, j == 0, j == k - 1, 0])
            nch += k
    q = 0
    while q < nch:
        r = chunk_info[q][0]
        n = 1
        while (q + n < nch and n < GMAX and chunk_info[q + n][0] == r):
            n += 1
        groups.append((r, q, n))
        for j in range(n):
            chunk_info[q + j][4] = j
        q += n
    chunk_info = tuple(tuple(x) for x in chunk_info)

    starts = np.zeros(NCORE * NR * nbk, dtype=np.int64)
    np.cumsum(counts.reshape(-1)[:-1], out=starts[1:])
    pos = np.arange(len(bkey_s), dtype=np.int64) - starts[bkey_s]
    core_s = core[order]
    q_of_edge = chunk_base[rng[order], blk[order]] + pos // 128
    p_of_edge = pos % 128

    eidx_arr = np.zeros((NCORE, 16, nch * 8), dtype=np.int16)
    erow_arr = np.zeros((NCORE, 128, nch), dtype=np.float32)
    eval_arr = np.zeros((NCORE, 128, nch), dtype=np.float32)
    eidx_arr[core_s, p_of_edge % 16, q_of_edge * 8 + p_of_edge // 16] = \
        col_local[order]
    erow_arr[core_s, p_of_edge, q_of_edge] = rowl[order]
    eval_arr[core_s, p_of_edge, q_of_edge] = vals_l[order]

    return (nch, tuple(groups), chunk_info,
            tuple(0 if x is None else x for x in first_r),
            tuple(0 if x is None else x for x in last_r),
            eidx_arr, erow_arr, eval_arr)


def _pack_inputs(userIdx, itemIdx, rows, cols, vals, uEmbd, iEmbd,
                 Wlin, blin, Wint, bint):
    rows = np.asarray(rows, dtype=np.int64)
    cols = np.asarray(cols, dtype=np.int64)
    vals = np.asarray(vals, dtype=np.float32)
    userIdx = np.asarray(userIdx, dtype=np.int64)
    itemIdx = np.asarray(itemIdx, dtype=np.int64)

    feat0 = np.ascontiguousarray(
        np.concatenate([np.asarray(uEmbd, np.float32),
                        np.asarray(iEmbd, np.float32)], axis=0).astype(BF))

    # ---- layer 1: host-pregathered stream, block-major by dest ----
    core0 = rows // SHARD
    local0 = rows - core0 * SHARD
    blk0 = local0 // RB
    rowl0 = (local0 - blk0 * RB).astype(np.float32)
    bkey0 = (core0 * NB + blk0).astype(np.int64)
    order0 = np.argsort(bkey0, kind="stable")
    counts0 = np.bincount(bkey0[order0], minlength=NCORE * NB)
    counts0 = counts0.reshape(NCORE, NB)
    k0 = np.ceil(counts0.max(axis=0) / 128).astype(np.int64)
    chunk0_l, base0, NCH0 = _chunks_of(counts0.max(axis=0), k0)
    base0_arr = np.zeros(NB, dtype=np.int64)
    for b, q in base0.items():
        base0_arr[b] = q
    starts0 = np.zeros(NCORE * NB, dtype=np.int64)
    np.cumsum(counts0.reshape(-1)[:-1], out=starts0[1:])
    pos0 = np.arange(len(order0), dtype=np.int64) - starts0[bkey0[order0]]
    core0_s = core0[order0]
    q0_of = base0_arr[blk0[order0]] + pos0 // 128
    p0_of = pos0 % 128

    col_of_slot = np.full((NCORE, 128, NCH0), -1, dtype=np.int64)
    col_of_slot[core0_s, p0_of, q0_of] = cols[order0]
    erow0_arr = np.zeros((NCORE, 128, NCH0), dtype=np.float32)
    eval0_arr = np.zeros((NCORE, 128, NCH0), dtype=np.float32)
    erow0_arr[core0_s, p0_of, q0_of] = rowl0[order0]
    eval0_arr[core0_s, p0_of, q0_of] = vals[order0]
    feat0z = np.concatenate([feat0, np.zeros((1, D), BF)], axis=0)
    # g0[c][p, q*128:(q+1)*128] = feat0[col_of_slot[c, p, q]]
    g0_arr = feat0z[col_of_slot]            # [NCORE, 128, NCH0, D]
    g0_arr = np.ascontiguousarray(
        g0_arr.reshape(NCORE, 128, NCH0 * D))

    # ---- layer 2: full shard, ranged gather from ags[0] ----
    (NCH1, groups1, chunk1, first_r1, last_r1,
     eidx1_arr, erow1_arr, eval1_arr) = _pack_spmm_ranged(
        rows, cols, vals, SHARD, NB)

    # ---- layer 3: pruned to referenced rows, re-sharded ----
    irow = itemIdx + NUM_USERS
    refset = np.union1d(userIdx, irow)       # sorted unique, ascending
    # per-core compact shard: contiguous split of the sorted referenced set
    nref = len(refset)
    # round-robin so each core's rows spread evenly over the int16 ranges
    core_of_ref = np.arange(nref) % NCORE
    # compute per-core per-range counts
    rng_ref = refset // RS
    cnt_cr = np.zeros((NCORE, NR), dtype=np.int64)
    for c in range(NCORE):
        cnt_cr[c] = np.bincount(rng_ref[core_of_ref == c], minlength=NR)
    F3R = (np.ceil(cnt_cr.max(axis=0) / 128) * 128).astype(np.int64)
    NF3T = int(F3R.sum())
    NS3 = NF3T
    NB3 = (NS3 + RB - 1) // RB
    rbase = np.zeros(NR, dtype=np.int64)
    np.cumsum(F3R[:-1], out=rbase[1:])
    # local position of each referenced row
    localpos = np.zeros(nref, dtype=np.int64)
    idxf3_arr = np.zeros((NCORE, 16, NF3T // 16), dtype=np.int16)
    for c in range(NCORE):
        m = core_of_ref == c
        rs = refset[m]
        rr = rng_ref[m]
        lp = np.zeros(len(rs), dtype=np.int64)
        for r in range(NR):
            mr = rr == r
            lp[mr] = rbase[r] + np.arange(int(mr.sum()))
        localpos[m] = lp
        # transposed-gather indices for layer-3 dense inputs (per range)
        jj = np.zeros(NF3T, dtype=np.int16)
        for r in range(NR):
            jr = np.zeros(F3R[r], dtype=np.int16)
            jr[:int(cnt_cr[c][r])] = (rs[rr == r] - r * RS).astype(np.int16)
            jj[rbase[r]:rbase[r] + F3R[r]] = jr
        idxf3_arr[c, np.arange(NF3T) % 16,
                  (np.arange(NF3T) // 128) * 8 +
                  (np.arange(NF3T) % 128) // 16] = jj
    compact = core_of_ref * NS3 + localpos   # global compact id per ref row
    cmap = np.full(N, -1, dtype=np.int64)
    cmap[refset] = compact

    keep = cmap[rows] >= 0
    (NCH2, groups2, chunk2, first_r2, last_r2,
     eidx2_arr, erow2_arr, eval2_arr) = _pack_spmm_ranged(
        cmap[rows[keep]], cols[keep], vals[keep], NS3, NB3)
    # note: transposed-gather ranges differ per range of SOURCE cols for
    # idxf3 (handled above); layer-3 gathers read full ags[1] (4 ranges).

    # ---- weights ----
    wlin_h = np.ascontiguousarray(
        np.asarray(Wlin, np.float32).transpose(1, 0, 2).reshape(D, NL * D)
        .astype(BF))
    wint_h = np.ascontiguousarray(
        np.asarray(Wint, np.float32).transpose(1, 0, 2).reshape(D, NL * D)
        .astype(BF))
    biasr = np.ascontiguousarray(
        (np.asarray(blin, np.float32) + np.asarray(bint, np.float32))
        .reshape(1, NL * D).astype(BF))

    # ---- final stage: bucket item rows by range (levels 0-2) ----
    ir = irow // RS
    nfb_counts = np.zeros((NCORE, NR), dtype=np.int64)
    perms = []
    for c in range(NCORE):
        sl = slice(c * BSH, (c + 1) * BSH)
        o = np.argsort(ir[sl], kind="stable")
        perms.append(o)
        nfb_counts[c] = np.bincount(ir[sl][o], minlength=NR)
    bucket_chunks = np.ceil(nfb_counts.max(axis=0) / 128).astype(np.int64)
    fin_bounds = []
    c0 = 0
    for r in range(NR):
        n = int(bucket_chunks[r])
        if n == 0:
            continue
        fin_bounds.append((r, c0, n))
        c0 += n
    NFB = c0

    uidx_arr = np.zeros((NCORE, 16, NFB * 8), dtype=np.int16)
    iidx_arr = np.zeros((NCORE, 16, NFB * 8), dtype=np.int16)
    uidx3_arr = np.zeros((NCORE, 16, NFB * 8), dtype=np.int16)
    iidx3_arr = np.zeros((NCORE, 16, NFB * 8), dtype=np.int16)
    inv_perm = np.full((NCORE, NFB * 128), -1, dtype=np.int64)
    for c in range(NCORE):
        sl = slice(c * BSH, (c + 1) * BSH)
        o = perms[c]
        u_s = userIdx[sl][o]
        i_s = irow[sl][o]
        r_s = ir[sl][o]
        jpos = np.zeros(BSH, dtype=np.int64)
        for (r, b0, nchk) in fin_bounds:
            m = r_s == r
            jpos[m] = b0 * 128 + np.arange(int(m.sum()))
        jp16 = jpos % 16
        jcol = (jpos // 128) * 8 + (jpos % 128) // 16
        uidx_arr[c, jp16, jcol] = u_s.astype(np.int16)
        iidx_arr[c, jp16, jcol] = (i_s - r_s * RS).astype(np.int16)
        uidx3_arr[c, jp16, jcol] = cmap[u_s].astype(np.int16)
        iidx3_arr[c, jp16, jcol] = cmap[i_s].astype(np.int16)
        inv_perm[c, jpos] = np.arange(c * BSH, (c + 1) * BSH)[o]

    meta = (NCH0, tuple(chunk0_l), NCH1, groups1, chunk1, first_r1,
            last_r1, NCH2, groups2, chunk2, first_r2, last_r2, NS3, NB3,
            NF3T, tuple(int(x) for x in F3R), tuple(fin_bounds), NFB)

    in_maps = []
    for c in range(NCORE):
        f0t = np.ascontiguousarray(feat0[c * SHARD:(c + 1) * SHARD].T)
        in_maps.append({
            "feat0": feat0,
            "f0t": f0t,
            "g0": g0_arr[c],
            "erow0": np.ascontiguousarray(erow0_arr[c]),
            "evals0": np.ascontiguousarray(eval0_arr[c]),
            "eidx1": np.ascontiguousarray(np.tile(eidx1_arr[c], (8, 1))),
            "erow1": np.ascontiguousarray(erow1_arr[c]),
            "evals1": np.ascontiguousarray(eval1_arr[c]),
            "eidx2": np.ascontiguousarray(np.tile(eidx2_arr[c], (8, 1))),
            "erow2": np.ascontiguousarray(erow2_arr[c]),
            "evals2": np.ascontiguousarray(eval2_arr[c]),
            "idxf3": np.ascontiguousarray(np.tile(idxf3_arr[c], (8, 1))),
            "wlin": wlin_h,
            "wint": wint_h,
            "biasr": biasr,
            "uidx": np.ascontiguousarray(np.tile(uidx_arr[c], (8, 1))),
            "iidx": np.ascontiguousarray(np.tile(iidx_arr[c], (8, 1))),
            "uidx3": np.ascontiguousarray(np.tile(uidx3_arr[c], (8, 1))),
            "iidx3": np.ascontiguousarray(np.tile(iidx3_arr[c], (8, 1))),
        })
    return meta, in_maps, inv_perm


def kernel(**inputs) -> np.ndarray:
    meta, in_maps, inv_perm = _pack_inputs(**inputs)
    key = _key(meta)
    if key not in _cache:
        _cache[key] = _build(meta)
    nc = _cache[key]
    res = run_bass_kernel_spmd(nc, in_maps, list(range(NCORE)))
    out = np.empty(BATCH, dtype=np.float32)
    NFB = meta[17]
    for c in range(NCORE):
        sc = res.results[c]["score"]  # [128, NFB]
        vals_j = sc[np.arange(NFB * 128) % 128, np.arange(NFB * 128) // 128]
        valid = inv_perm[c] >= 0
        out[inv_perm[c][valid]] = vals_j[valid]
    return out


# revision 76
# speedup vs baseline: 1.0184x; 1.0184x over previous
"""GCF (graph collaborative filtering) message passing on 8 Trainium2 cores.

Sharding: nodes split contiguously for layers 1-2 (core c owns rows
[c*12500, (c+1)*12500)); layer 3 computes only the rows referenced by the
final batch gather (~27k of 100k), re-sharded evenly across cores.

Per layer SpMM: edges bucketed by dest 256-row block; per 128-edge chunk a
one-hot O[e, row] = vals[e]*(rowl[e]==row) is built in one vector
tensor_scalar, and PE accumulates Lx^T += Xg^T @ O per block in PSUM.
  - Layer 1 sources come from the input table, so the host pre-gathers
    feat0[cols] into a partition-major bf16 stream loaded with plain
    dma_start (no gpsimd descriptor generation); chunks are block-major and
    the dense pass-1 for a block runs as soon as its last chunk lands.
  - Layers 2-3 dma_gather from the AllGather table (4 int16 ranges of 32768
    rows); Lx spills to an SBUF accumulator across ranges.
Dense (two passes, transposed layout): pass 1 per block computes
y^T = Wlin^T(Lx+F)^T + Wint^T(Lx*F)^T in PSUM, fused bias+leaky-relu into
the feature buffer, and a ones-matmul accumulates per-row square-norms into
one [NB,256] PSUM tile. Pass 2 does a single sqrt + reciprocal for all
blocks, then per block a K=1 broadcast matmul + multiply normalizes, and PE
transposes write the normal-layout shard for the AllGather.

Final: batch split 2048/core; per concat level dma_gather u/i rows,
multiply+reduce into an accumulator; levels are emitted as soon as their
table is available so the gathers overlap later sweeps. Host inverts the
slot permutation. Level 3 reads the compact pruned table (single range).

All feature data is bf16 (PSUM accumulation fp32); norms fp32.
"""

import os

import ml_dtypes
import numpy as np

import concourse.bacc as bacc
import concourse.mybir as mybir
import concourse.tile as tile
from concourse.bass import ts
from concourse.bass_utils import run_bass_kernel_spmd
from concourse.masks import make_identity

BF = ml_dtypes.bfloat16

NUM_USERS = 30000
NUM_ITEMS = 70000
N = 100000
D = 128
NL = 3
BATCH = 16384
NCORE = 8
SHARD = N // NCORE            # 12500
RB = 256                      # dest rows per block
NB = (SHARD + RB - 1) // RB   # 49 blocks/core (layers 1-2)
RS = 32768                    # range size (int16 index window)
NR = 4                        # ranges
GMAX = 24                     # max chunks per dma_gather call
SGC = 16                      # chunks per layer-1 stream tile
SGD = 8                       # chunks per layer-1 stream dma_start
BSH = BATCH // NCORE          # 2048
EPS2 = 1e-24
SLOPE = 0.01
SP = os.environ.get("KSP", "0") == "1"

f32 = mybir.dt.float32
f32r = mybir.dt.float32r
bf16 = mybir.dt.bfloat16
i16 = mybir.dt.int16

_cache = {}


def _key(meta):
    return repr(meta)


def _build(meta):
    (NCH0, chunk0, NCH1, groups1, chunk1, first_r1, last_r1,
     NCH2, groups2, chunk2, first_r2, last_r2, NS3, NB3, NF3T, F3R,
     fin_bounds, NFB) = meta
    nc = bacc.Bacc(num_devices=NCORE, num_swdge_queues=4)

    feat0 = nc.dram_tensor("feat0", [N, D], bf16, kind="ExternalInput")
    f0t = nc.dram_tensor("f0t", [D, SHARD], bf16, kind="ExternalInput")
    g0 = nc.dram_tensor("g0", [128, NCH0 * 128], bf16, kind="ExternalInput")
    erow0 = nc.dram_tensor("erow0", [128, NCH0], f32, kind="ExternalInput")
    evals0 = nc.dram_tensor("evals0", [128, NCH0], f32, kind="ExternalInput")
    eidx1 = nc.dram_tensor("eidx1", [128, NCH1 * 8], i16, kind="ExternalInput")
    erow1 = nc.dram_tensor("erow1", [128, NCH1], f32, kind="ExternalInput")
    evals1 = nc.dram_tensor("evals1", [128, NCH1], f32, kind="ExternalInput")
    eidx2 = nc.dram_tensor("eidx2", [128, NCH2 * 8], i16, kind="ExternalInput")
    erow2 = nc.dram_tensor("erow2", [128, NCH2], f32, kind="ExternalInput")
    evals2 = nc.dram_tensor("evals2", [128, NCH2], f32, kind="ExternalInput")
    idxf3 = nc.dram_tensor("idxf3", [128, NF3T // 16], i16,
                           kind="ExternalInput")
    wlin = nc.dram_tensor("wlin", [D, NL * D], bf16, kind="ExternalInput")
    wint = nc.dram_tensor("wint", [D, NL * D], bf16, kind="ExternalInput")
    biasc = nc.dram_tensor("biasc", [D, NL], f32, kind="ExternalInput")
    uidx = nc.dram_tensor("uidx", [128, NFB * 8], i16, kind="ExternalInput")
    iidx = nc.dram_tensor("iidx", [128, NFB * 8], i16, kind="ExternalInput")
    uidx3 = nc.dram_tensor("uidx3", [128, NFB * 8], i16, kind="ExternalInput")
    iidx3 = nc.dram_tensor("iidx3", [128, NFB * 8], i16, kind="ExternalInput")
    score = nc.dram_tensor("score", [128, NFB], f32, kind="ExternalOutput")

    add = mybir.AluOpType.add
    mult = mybir.AluOpType.mult
    is_equal = mybir.AluOpType.is_equal
    AF = mybir.ActivationFunctionType

    with tile.TileContext(nc) as tc:
        with (
            tc.tile_pool(name="const", bufs=1) as cp,
            tc.tile_pool(name="ft", bufs=1) as ftp,
            tc.tile_pool(name="st", bufs=3) as stp,
            tc.tile_pool(name="g", bufs=3) as gp,
            tc.tile_pool(name="fin", bufs=1) as fp_,
            tc.tile_pool(name="ot", bufs=8) as otp,
            tc.tile_pool(name="sb", bufs=3) as sbp,
            tc.tile_pool(name="plx", bufs=3, space="PSUM") as plx,
            tc.tile_pool(name="py", bufs=3, space="PSUM") as pyp,
            tc.tile_pool(name="ptp", bufs=2, space="PSUM") as ptp,
            tc.tile_pool(name="dram", bufs=1, space="DRAM") as dp,
        ):
            # ---------- constants ----------
            iota = cp.tile([128, RB], bf16)
            nc.gpsimd.iota(iota[:], pattern=[[1, RB]], base=0,
                           channel_multiplier=0,
                           allow_small_or_imprecise_dtypes=True)
            ident = cp.tile([128, 128], bf16)
            make_identity(nc, ident[:])
            epsb = cp.tile([128, 1], f32)
            nc.vector.memset(epsb[:], EPS2)

            wlin_b = cp.tile([128, NL * 128], bf16)
            nc.sync.dma_start(out=wlin_b[:], in_=wlin[:])
            wint_b = cp.tile([128, NL * 128], bf16)
            nc.sync.dma_start(out=wint_b[:], in_=wint[:])
            bias_sb = cp.tile([128, NL], f32)
            nc.sync.dma_start(out=bias_sb[:], in_=biasc[:])

            erow0_sb = cp.tile([128, NCH0], f32)
            nc.sync.dma_start(out=erow0_sb[:], in_=erow0[:])
            evals0_sb = cp.tile([128, NCH0], f32)
            nc.sync.dma_start(out=evals0_sb[:], in_=evals0[:])
            eidx1_sb = cp.tile([128, NCH1 * 8], i16)
            nc.sync.dma_start(out=eidx1_sb[:], in_=eidx1[:])
            erow1_sb = cp.tile([128, NCH1], f32)
            nc.sync.dma_start(out=erow1_sb[:], in_=erow1[:])
            evals1_sb = cp.tile([128, NCH1], f32)
            nc.sync.dma_start(out=evals1_sb[:], in_=evals1[:])
            eidx2_sb = cp.tile([128, NCH2 * 8], i16)
            nc.sync.dma_start(out=eidx2_sb[:], in_=eidx2[:])
            erow2_sb = cp.tile([128, NCH2], f32)
            nc.sync.dma_start(out=erow2_sb[:], in_=erow2[:])
            evals2_sb = cp.tile([128, NCH2], f32)
            nc.sync.dma_start(out=evals2_sb[:], in_=evals2[:])
            idxf3_sb = cp.tile([128, NF3T // 16], i16)
            nc.sync.dma_start(out=idxf3_sb[:], in_=idxf3[:])
            uidx_sb = cp.tile([128, NFB * 8], i16)
            nc.sync.dma_start(out=uidx_sb[:], in_=uidx[:])
            iidx_sb = cp.tile([128, NFB * 8], i16)
            nc.sync.dma_start(out=iidx_sb[:], in_=iidx[:])
            uidx3_sb = cp.tile([128, NFB * 8], i16)
            nc.sync.dma_start(out=uidx3_sb[:], in_=uidx3[:])
            iidx3_sb = cp.tile([128, NFB * 8], i16)
            nc.sync.dma_start(out=iidx3_sb[:], in_=iidx3[:])

            fta = ftp.tile([128, NB * RB], bf16, tag="fta")
            ftb = ftp.tile([128, NB * RB], bf16, tag="ftb")
            ft3in = ftp.tile([128, NB3 * RB], bf16, tag="ft3in")
            ft3out = ftp.tile([128, NB3 * RB], bf16, tag="ft3out")
            nc.vector.memset(fta[:, SHARD:], 0.0)
            nc.vector.memset(ft3out[:], 0.0)
            nc.vector.memset(ft3in[:], 0.0)
            nc.sync.dma_start(out=fta[:, :SHARD], in_=f0t[:])

            fshard = dp.tile([NB * RB, D], bf16)
            fshard3 = dp.tile([NS3, D], bf16)
            ags = [dp.tile([N, D], bf16, name=f"ag{i}", tag=f"ag{i}",
                           addr_space="Shared") for i in range(2)]
            ag3 = dp.tile([NCORE * NS3, D], bf16, name="ag3", tag="ag3",
                          addr_space="Shared")

            acc = cp.tile([128, NFB], f32)

            # ---------- helpers ----------
            def onehot(q, qrow, qval, tag):
                o = otp.tile([128, RB], bf16, tag="o", name=f"o_{tag}_{q}")
                nc.vector.tensor_scalar(
                    out=o[:], in0=iota[:],
                    scalar1=qrow[:, q:q + 1],
                    scalar2=qval[:, q:q + 1],
                    op0=is_equal, op1=mult,
                )
                return o

            def pass1(l, b, ftin, ftout):
                lxs = ftout[:, ts(b, RB)]
                fin_ = ftin[:, ts(b, RB)]
                pre1 = sbp.tile([128, RB], bf16, tag="pre1")
                nc.vector.tensor_tensor(out=pre1[:], in0=lxs, in1=fin_,
                                        op=add)
                pre2 = sbp.tile([128, RB], bf16, tag="pre2")
                nc.vector.tensor_tensor(out=pre2[:], in0=lxs, in1=fin_,
                                        op=mult)
                y = pyp.tile([128, RB], f32, tag="y")
                nc.tensor.matmul(out=y[:], lhsT=wlin_b[:, ts(l, 128)],
                                 rhs=pre1[:], start=True, stop=False)
                nc.tensor.matmul(out=y[:], lhsT=wint_b[:, ts(l, 128)],
                                 rhs=pre2[:], start=False, stop=True)
                nc.scalar.activation(out=lxs, in_=y[:], func=AF.Lrelu,
                                     bias=bias_sb[:, l:l + 1], scale=1.0,
                                     alpha=SLOPE)

            def pass2(l, nb, nrows, ftout, fsh):
                # normalize in normal layout: transpose, per-row norm,
                # fused scale into the shard-write copy
                for b in range(nb):
                    for h in range(2):
                        r0 = b * RB + h * 128
                        nr = min(128, nrows - r0)
                        if nr <= 0:
                            break
                        tp = ptp.tile([128, 128], bf16, tag="tp",
                                      name=f"tp{l}_{b}_{h}")
                        nc.tensor.transpose(
                            out=tp[:], in_=ftout[:, r0:r0 + 128],
                            identity=ident[:])
                        tpc = sbp.tile([128, 128], bf16, tag="tpc")
                        nc.vector.tensor_copy(tpc[:], tp[:])
                        sq2 = sbp.tile([128, 128], f32, tag="sq2")
                        nc.vector.tensor_tensor(out=sq2[:], in0=tpc[:],
                                                in1=tpc[:], op=mult)
                        ssum = sbp.tile([128, 1], f32, tag="ssum")
                        nc.vector.tensor_reduce(
                            out=ssum[:], in_=sq2[:],
                            axis=mybir.AxisListType.X, op=add)
                        rt2 = sbp.tile([128, 1], f32, tag="rt2")
                        nc.scalar.activation(out=rt2[:], in_=ssum[:],
                                             func=AF.Sqrt, bias=epsb[:])
                        inv1 = sbp.tile([128, 1], f32, tag="inv1")
                        nc.vector.reciprocal(inv1[:], rt2[:])
                        cpo = sbp.tile([128, 128], bf16, tag="cpo")
                        nc.vector.tensor_scalar(
                            out=cpo[:], in0=tpc[:], scalar1=inv1[:],
                            scalar2=None, op0=mult)
                        nc.sync.dma_start(out=fsh[r0:r0 + nr, :],
                                          in_=cpo[:nr, :])

            def final_level(li, srcf, u_sb, i_sb, bounds):
                ug = fp_.tile([128, NFB * 128], bf16, tag="ug",
                              name=f"ug{li}")
                nc.gpsimd.dma_gather(
                    ug[:].rearrange("p (c d) -> p c d", d=128),
                    srcf[:],
                    u_sb[:],
                    NFB * 128, NFB * 128, 128,
                    single_packet=SP,
                    queue_num=3,
                )
                ig = fp_.tile([128, NFB * 128], bf16, tag="ig",
                              name=f"ig{li}")
                for qi, (rr, c0, cn) in enumerate(bounds):
                    nc.gpsimd.dma_gather(
                        ig[:, c0 * 128:(c0 + cn) * 128].rearrange(
                            "p (c d) -> p c d", d=128),
                        srcf[rr * RS:, :],
                        i_sb[:, c0 * 8:(c0 + cn) * 8],
                        cn * 128, cn * 128, 128,
                        single_packet=SP,
                        queue_num=qi % 4,
                    )
                nc.vector.tensor_tensor(out=ug[:], in0=ug[:], in1=ig[:],
                                        op=mult)
                sc = sbp.tile([128, NFB], f32, tag="sc")
                nc.vector.tensor_reduce(
                    out=sc[:],
                    in_=ug[:].rearrange("p (c d) -> p c d", d=128),
                    axis=mybir.AxisListType.X, op=add)
                if li == 0:
                    nc.vector.tensor_copy(acc[:], sc[:])
                else:
                    nc.vector.tensor_tensor(out=acc[:], in0=acc[:],
                                            in1=sc[:], op=add)

            # ---------- level-0 final gather (overlaps layer-1 sweep) ----
            final_level(0, feat0, uidx_sb, iidx_sb, fin_bounds)

            # ---------- layer 1: streamed sources, block-major ----------
            cur_lx = None
            nst = (NCH0 + SGC - 1) // SGC
            for si in range(nst):
                q0 = si * SGC
                gn = min(SGC, NCH0 - q0)
                st = stp.tile([128, SGC * 128], bf16, tag="st",
                              name=f"st{si}")
                for j in range(0, gn, SGD):
                    jn = min(SGD, gn - j)
                    nc.sync.dma_start(
                        out=st[:, j * 128:(j + jn) * 128],
                        in_=g0[:, (q0 + j) * 128:(q0 + j + jn) * 128])
                for q in range(q0, q0 + gn):
                    b_q, is_first, is_last = chunk0[q]
                    o = onehot(q, erow0_sb, evals0_sb, "l0")
                    if is_first:
                        cur_lx = plx.tile([128, RB], f32, tag="lx",
                                          name=f"lx0_{q}")
                    nc.tensor.matmul(
                        out=cur_lx[:],
                        lhsT=st[:, ts(q - q0, 128)],
                        rhs=o[:],
                        start=is_first, stop=is_last,
                    )
                    if is_last:
                        nc.vector.tensor_copy(ftb[:, ts(b_q, RB)],
                                              cur_lx[:])
                        pass1(0, b_q, fta, ftb)
            pass2(0, NB, NB * RB, ftb, fshard)
            # reload normalized features, transposed, as layer-2 dense input
            nc.sync.dma_start_transpose(out=fta[:], in_=fshard[:])
            nc.gpsimd.collective_compute(
                "AllGather", mybir.AluOpType.bypass,
                replica_groups=[list(range(NCORE))],
                ins=[fshard[:SHARD, :].opt()], outs=[ags[0].opt()],
            )

            # ---------- layers 2-3: gathered sources, range-major --------
            for l, (NCHl, groups, chunk_info, first_r, last_r, eidx_sb,
                    erow_sb, evals_sb, src, ftin, ftout, nb, nrows, fsh,
                    agout) in enumerate((
                (NCH1, groups1, chunk1, first_r1, last_r1, eidx1_sb,
                 erow1_sb, evals1_sb, ags[0], fta, ftb, NB, NB * RB,
                 fshard, ags[1]),
                (NCH2, groups2, chunk2, first_r2, last_r2, eidx2_sb,
                 erow2_sb, evals2_sb, ags[1], ft3in, ft3out, NB3, NS3,
                 fshard3, ag3),
            ), start=1):
                cur_lx = None
                for gi, (gr, q0, gn) in enumerate(groups):
                    gt = gp.tile([128, GMAX * 128], bf16, tag="g",
                                 name=f"g{l}_{gi}")
                    nc.gpsimd.dma_gather(
                        gt[:, :gn * 128].rearrange("p (c d) -> p c d",
                                                   d=128),
                        src[gr * RS:, :],
                        eidx_sb[:, q0 * 8:(q0 + gn) * 8],
                        gn * 128, gn * 128, 128,
                        single_packet=SP,
                        queue_num=gi % 4,
                    )
                    for q in range(q0, q0 + gn):
                        r_q, b_q, is_first, is_last, loc = chunk_info[q]
                        o = onehot(q, erow_sb, evals_sb, f"l{l}")
                        if is_first:
                            cur_lx = plx.tile([128, RB], f32, tag="lx",
                                              name=f"lx{l}_{q}")
                        nc.tensor.matmul(
                            out=cur_lx[:],
                            lhsT=gt[:, ts(loc, 128)],
                            rhs=o[:],
                            start=is_first, stop=is_last,
                        )
                        if is_last:
                            dst = ftout[:, ts(b_q, RB)]
                            if first_r[b_q] == r_q:
                                nc.scalar.copy(dst, cur_lx[:])
                            else:
                                nc.vector.tensor_tensor(
                                    out=dst, in0=dst, in1=cur_lx[:],
                                    op=add)
                            if last_r[b_q] == r_q:
                                pass1(l, b_q, ftin, ftout)
                # final-level gathers fill the gpsimd window while the
                # dense pass drains (their table came from the previous CC)
                final_level(l, ags[l - 1], uidx_sb, iidx_sb, fin_bounds)
                pass2(l, nb, nrows, ftout, fsh)
                nc.gpsimd.collective_compute(
                    "AllGather", mybir.AluOpType.bypass,
                    replica_groups=[list(range(NCORE))],
                    ins=[fsh[:SHARD, :].opt() if l == 1 else fsh.opt()],
                    outs=[agout.opt()],
                )
                if l == 1:
                    # layer-3 dense inputs: transposed gather of the compact
                    # rows' layer-2 features (per int16 source range)
                    rb0 = 0
                    for r in range(NR):
                        fn = F3R[r]
                        if fn == 0:
                            continue
                        nc.gpsimd.dma_gather(
                            ft3in[:, rb0:rb0 + fn].rearrange(
                                "p (c d) -> p c d", c=1),
                            ags[1][r * RS:, :],
                            idxf3_sb[:, rb0 // 16:(rb0 + fn) // 16],
                            fn, fn, 128,
                            transpose=True,
                            single_packet=SP,
                            queue_num=r % 4,
                        )
                        rb0 += fn
                else:
                    final_level(3, ag3, uidx3_sb, iidx3_sb,
                                [(0, 0, NFB)])
            nc.sync.dma_start(out=score[:], in_=acc[:])

    nc.compile()
    return nc


def _chunks_of(counts, kk):
    """Block-major chunk table: [(b, is_first, is_last)] and base offsets."""
    info = []
    base = {}
    q = 0
    for b, k in enumerate(kk):
        if k == 0:
            continue
        base[b] = q
        for j in range(k):
            info.append((b, j == 0, j == k - 1))
        q += k
    return info, base, q


def _pack_spmm_ranged(rows_l, cols_l, vals_l, nshard, nblocks):
    """Bucket edges by (core, range, block) with shared chunk counts.

    rows_l are destination indices in the (possibly compact) shard space
    (core = row // nshard); cols_l index the source table (range split).
    Returns per-core eidx/erow/evals arrays plus chunk/group metadata.
    """
    core = rows_l // nshard
    local = rows_l - core * nshard
    blk = local // RB
    rowl = (local - blk * RB).astype(np.float32)
    rng = cols_l // RS
    col_local = (cols_l - rng * RS).astype(np.int16)

    nbk = nblocks
    bkey = ((core * NR + rng) * nbk + blk).astype(np.int64)
    order = np.argsort(bkey, kind="stable")
    bkey_s = bkey[order]
    counts = np.bincount(bkey_s, minlength=NCORE * NR * nbk)
    counts = counts.reshape(NCORE, NR, nbk)
    k2 = np.ceil(counts.max(axis=0) / 128).astype(np.int64)  # [NR, nbk]

    chunk_base = np.zeros((NR, nbk), dtype=np.int64)
    nch = 0
    chunk_info = []          # (r, b, is_first, is_last, loc)
    groups = []              # (r, q0, n)
    first_r = [None] * nbk
    last_r = [None] * nbk
    for r in range(NR):
        for b in range(nbk):
            k = int(k2[r, b])
            if k == 0:
                continue
            if first_r[b] is None:
                first_r[b] = r
            last_r[b] = r
            chunk_base[r, b] = nch
            for j in range(k):
                chunk_info.append([r, b, j == 0, j == k - 1, 0])
            nch += k
    q = 0
    while q < nch:
        r = chunk_info[q][0]
        n = 1
        while (q + n < nch and n < GMAX and chunk_info[q + n][0] == r):
            n += 1
        groups.append((r, q, n))
        for j in range(n):
            chunk_info[q + j][4] = j
        q += n
    chunk_info = tuple(tuple(x) for x in chunk_info)

    starts = np.zeros(NCORE * NR * nbk, dtype=np.int64)
    np.cumsum(counts.reshape(-1)[:-1], out=starts[1:])
    pos = np.arange(len(bkey_s), dtype=np.int64) - starts[bkey_s]
    core_s = core[order]
    q_of_edge = chunk_base[rng[order], blk[order]] + pos // 128
    p_of_edge = pos % 128

    eidx_arr = np.zeros((NCORE, 16, nch * 8), dtype=np.int16)
    erow_arr = np.zeros((NCORE, 128, nch), dtype=np.float32)
    eval_arr = np.zeros((NCORE, 128, nch), dtype=np.float32)
    eidx_arr[core_s, p_of_edge % 16, q_of_edge * 8 + p_of_edge // 16] = \
        col_local[order]
    erow_arr[core_s, p_of_edge, q_of_edge] = rowl[order]
    eval_arr[core_s, p_of_edge, q_of_edge] = vals_l[order]

    return (nch, tuple(groups), chunk_info,
            tuple(0 if x is None else x for x in first_r),
            tuple(0 if x is None else x for x in last_r),
            eidx_arr, erow_arr, eval_arr)


def _pack_inputs(userIdx, itemIdx, rows, cols, vals, uEmbd, iEmbd,
                 Wlin, blin, Wint, bint):
    rows = np.asarray(rows, dtype=np.int64)
    cols = np.asarray(cols, dtype=np.int64)
    vals = np.asarray(vals, dtype=np.float32)
    userIdx = np.asarray(userIdx, dtype=np.int64)
    itemIdx = np.asarray(itemIdx, dtype=np.int64)

    feat0 = np.ascontiguousarray(
        np.concatenate([np.asarray(uEmbd, np.float32),
                        np.asarray(iEmbd, np.float32)], axis=0).astype(BF))

    # ---- layer 1: host-pregathered stream, block-major by dest ----
    core0 = rows // SHARD
    local0 = rows - core0 * SHARD
    blk0 = local0 // RB
    rowl0 = (local0 - blk0 * RB).astype(np.float32)
    bkey0 = (core0 * NB + blk0).astype(np.int64)
    order0 = np.argsort(bkey0, kind="stable")
    counts0 = np.bincount(bkey0[order0], minlength=NCORE * NB)
    counts0 = counts0.reshape(NCORE, NB)
    k0 = np.ceil(counts0.max(axis=0) / 128).astype(np.int64)
    chunk0_l, base0, NCH0 = _chunks_of(counts0.max(axis=0), k0)
    base0_arr = np.zeros(NB, dtype=np.int64)
    for b, q in base0.items():
        base0_arr[b] = q
    starts0 = np.zeros(NCORE * NB, dtype=np.int64)
    np.cumsum(counts0.reshape(-1)[:-1], out=starts0[1:])
    pos0 = np.arange(len(order0), dtype=np.int64) - starts0[bkey0[order0]]
    core0_s = core0[order0]
    q0_of = base0_arr[blk0[order0]] + pos0 // 128
    p0_of = pos0 % 128

    col_of_slot = np.full((NCORE, 128, NCH0), -1, dtype=np.int64)
    col_of_slot[core0_s, p0_of, q0_of] = cols[order0]
    erow0_arr = np.zeros((NCORE, 128, NCH0), dtype=np.float32)
    eval0_arr = np.zeros((NCORE, 128, NCH0), dtype=np.float32)
    erow0_arr[core0_s, p0_of, q0_of] = rowl0[order0]
    eval0_arr[core0_s, p0_of, q0_of] = vals[order0]
    feat0z = np.concatenate([feat0, np.zeros((1, D), BF)], axis=0)
    # g0[c][p, q*128:(q+1)*128] = feat0[col_of_slot[c, p, q]]
    g0_arr = feat0z[col_of_slot]            # [NCORE, 128, NCH0, D]
    g0_arr = np.ascontiguousarray(
        g0_arr.reshape(NCORE, 128, NCH0 * D))

    # ---- layer 2: full shard, ranged gather from ags[0] ----
    (NCH1, groups1, chunk1, first_r1, last_r1,
     eidx1_arr, erow1_arr, eval1_arr) = _pack_spmm_ranged(
        rows, cols, vals, SHARD, NB)

    # ---- layer 3: pruned to referenced rows, re-sharded ----
    irow = itemIdx + NUM_USERS
    refset = np.union1d(userIdx, irow)       # sorted unique, ascending
    # per-core compact shard: contiguous split of the sorted referenced set
    nref = len(refset)
    # round-robin so each core's rows spread evenly over the int16 ranges
    core_of_ref = np.arange(nref) % NCORE
    # compute per-core per-range counts
    rng_ref = refset // RS
    cnt_cr = np.zeros((NCORE, NR), dtype=np.int64)
    for c in range(NCORE):
        cnt_cr[c] = np.bincount(rng_ref[core_of_ref == c], minlength=NR)
    F3R = (np.ceil(cnt_cr.max(axis=0) / 128) * 128).astype(np.int64)
    NF3T = int(F3R.sum())
    NS3 = NF3T
    NB3 = (NS3 + RB - 1) // RB
    rbase = np.zeros(NR, dtype=np.int64)
    np.cumsum(F3R[:-1], out=rbase[1:])
    # local position of each referenced row
    localpos = np.zeros(nref, dtype=np.int64)
    idxf3_arr = np.zeros((NCORE, 16, NF3T // 16), dtype=np.int16)
    for c in range(NCORE):
        m = core_of_ref == c
        rs = refset[m]
        rr = rng_ref[m]
        lp = np.zeros(len(rs), dtype=np.int64)
        for r in range(NR):
            mr = rr == r
            lp[mr] = rbase[r] + np.arange(int(mr.sum()))
        localpos[m] = lp
        # transposed-gather indices for layer-3 dense inputs (per range)
        jj = np.zeros(NF3T, dtype=np.int16)
        for r in range(NR):
            jr = np.zeros(F3R[r], dtype=np.int16)
            jr[:int(cnt_cr[c][r])] = (rs[rr == r] - r * RS).astype(np.int16)
            jj[rbase[r]:rbase[r] + F3R[r]] = jr
        idxf3_arr[c, np.arange(NF3T) % 16,
                  (np.arange(NF3T) // 128) * 8 +
                  (np.arange(NF3T) % 128) // 16] = jj
    compact = core_of_ref * NS3 + localpos   # global compact id per ref row
    cmap = np.full(N, -1, dtype=np.int64)
    cmap[refset] = compact

    keep = cmap[rows] >= 0
    (NCH2, groups2, chunk2, first_r2, last_r2,
     eidx2_arr, erow2_arr, eval2_arr) = _pack_spmm_ranged(
        cmap[rows[keep]], cols[keep], vals[keep], NS3, NB3)
    # note: transposed-gather ranges differ per range of SOURCE cols for
    # idxf3 (handled above); layer-3 gathers read full ags[1] (4 ranges).

    # ---- weights ----
    wlin_h = np.ascontiguousarray(
        np.asarray(Wlin, np.float32).transpose(1, 0, 2).reshape(D, NL * D)
        .astype(BF))
    wint_h = np.ascontiguousarray(
        np.asarray(Wint, np.float32).transpose(1, 0, 2).reshape(D, NL * D)
        .astype(BF))
    biasc = np.ascontiguousarray(
        (np.asarray(blin, np.float32) + np.asarray(bint, np.float32)).T)

    # ---- final stage: bucket item rows by range (levels 0-2) ----
    ir = irow // RS
    nfb_counts = np.zeros((NCORE, NR), dtype=np.int64)
    perms = []
    for c in range(NCORE):
        sl = slice(c * BSH, (c + 1) * BSH)
        o = np.argsort(ir[sl], kind="stable")
        perms.append(o)
        nfb_counts[c] = np.bincount(ir[sl][o], minlength=NR)
    bucket_chunks = np.ceil(nfb_counts.max(axis=0) / 128).astype(np.int64)
    fin_bounds = []
    c0 = 0
    for r in range(NR):
        n = int(bucket_chunks[r])
        if n == 0:
            continue
        fin_bounds.append((r, c0, n))
        c0 += n
    NFB = c0

    uidx_arr = np.zeros((NCORE, 16, NFB * 8), dtype=np.int16)
    iidx_arr = np.zeros((NCORE, 16, NFB * 8), dtype=np.int16)
    uidx3_arr = np.zeros((NCORE, 16, NFB * 8), dtype=np.int16)
    iidx3_arr = np.zeros((NCORE, 16, NFB * 8), dtype=np.int16)
    inv_perm = np.full((NCORE, NFB * 128), -1, dtype=np.int64)
    for c in range(NCORE):
        sl = slice(c * BSH, (c + 1) * BSH)
        o = perms[c]
        u_s = userIdx[sl][o]
        i_s = irow[sl][o]
        r_s = ir[sl][o]
        jpos = np.zeros(BSH, dtype=np.int64)
        for (r, b0, nchk) in fin_bounds:
            m = r_s == r
            jpos[m] = b0 * 128 + np.arange(int(m.sum()))
        jp16 = jpos % 16
        jcol = (jpos // 128) * 8 + (jpos % 128) // 16
        uidx_arr[c, jp16, jcol] = u_s.astype(np.int16)
        iidx_arr[c, jp16, jcol] = (i_s - r_s * RS).astype(np.int16)
        uidx3_arr[c, jp16, jcol] = cmap[u_s].astype(np.int16)
        iidx3_arr[c, jp16, jcol] = cmap[i_s].astype(np.int16)
        inv_perm[c, jpos] = np.arange(c * BSH, (c + 1) * BSH)[o]

    meta = (NCH0, tuple(chunk0_l), NCH1, groups1, chunk1, first_r1,
            last_r1, NCH2, groups2, chunk2, first_r2, last_r2, NS3, NB3,
            NF3T, tuple(int(x) for x in F3R), tuple(fin_bounds), NFB)

    in_maps = []
    for c in range(NCORE):
        f0t = np.ascontiguousarray(feat0[c * SHARD:(c + 1) * SHARD].T)
        in_maps.append({
            "feat0": feat0,
            "f0t": f0t,
            "g0": g0_arr[c],
            "erow0": np.ascontiguousarray(erow0_arr[c]),
            "evals0": np.ascontiguousarray(eval0_arr[c]),
            "eidx1": np.ascontiguousarray(np.tile(eidx1_arr[c], (8, 1))),
            "erow1": np.ascontiguousarray(erow1_arr[c]),
            "evals1": np.ascontiguousarray(eval1_arr[c]),
            "eidx2": np.ascontiguousarray(np.tile(eidx2_arr[c], (8, 1))),
            "erow2": np.ascontiguousarray(erow2_arr[c]),
            "evals2": np.ascontiguousarray(eval2_arr[c]),
            "idxf3": np.ascontiguousarray(np.tile(idxf3_arr[c], (8, 1))),
            "wlin": wlin_h,
            "wint": wint_h,
            "biasc": biasc,
            "uidx": np.ascontiguousarray(np.tile(uidx_arr[c], (8, 1))),
            "iidx": np.ascontiguousarray(np.tile(iidx_arr[c], (8, 1))),
            "uidx3": np.ascontiguousarray(np.tile(uidx3_arr[c], (8, 1))),
            "iidx3": np.ascontiguousarray(np.tile(iidx3_arr[c], (8, 1))),
        })
    return meta, in_maps, inv_perm


def kernel(**inputs) -> np.ndarray:
    meta, in_maps, inv_perm = _pack_inputs(**inputs)
    key = _key(meta)
    if key not in _cache:
        _cache[key] = _build(meta)
    nc = _cache[key]
    res = run_bass_kernel_spmd(nc, in_maps, list(range(NCORE)))
    out = np.empty(BATCH, dtype=np.float32)
    NFB = meta[17]
    for c in range(NCORE):
        sc = res.results[c]["score"]  # [128, NFB]
        vals_j = sc[np.arange(NFB * 128) % 128, np.arange(NFB * 128) // 128]
        valid = inv_perm[c] >= 0
        out[inv_perm[c][valid]] = vals_j[valid]
    return out


# revision 79
# speedup vs baseline: 1.0260x; 1.0075x over previous
"""GCF (graph collaborative filtering) message passing on 8 Trainium2 cores.

Sharding: nodes split contiguously for layers 1-2 (core c owns rows
[c*12500, (c+1)*12500)); layer 3 computes only the rows referenced by the
final batch gather (~27k of 100k), re-sharded evenly across cores.

Per layer SpMM: edges bucketed by dest 256-row block; per 128-edge chunk a
one-hot O[e, row] = vals[e]*(rowl[e]==row) is built in one vector
tensor_scalar, and PE accumulates Lx^T += Xg^T @ O per block in PSUM.
  - Layer 1 sources come from the input table, so the host pre-gathers
    feat0[cols] into a partition-major bf16 stream loaded with plain
    dma_start (no gpsimd descriptor generation); chunks are block-major and
    the dense pass-1 for a block runs as soon as its last chunk lands.
  - Layers 2-3 dma_gather from the AllGather table (4 int16 ranges of 32768
    rows); Lx spills to an SBUF accumulator across ranges.
Dense (two passes, transposed layout): pass 1 per block computes
y^T = Wlin^T(Lx+F)^T + Wint^T(Lx*F)^T in PSUM, fused bias+leaky-relu into
the feature buffer, and a ones-matmul accumulates per-row square-norms into
one [NB,256] PSUM tile. Pass 2 does a single sqrt + reciprocal for all
blocks, then per block a K=1 broadcast matmul + multiply normalizes, and PE
transposes write the normal-layout shard for the AllGather.

Final: batch split 2048/core; per concat level dma_gather u/i rows,
multiply+reduce into an accumulator; levels are emitted as soon as their
table is available so the gathers overlap later sweeps. Host inverts the
slot permutation. Level 3 reads the compact pruned table (single range).

All feature data is bf16 (PSUM accumulation fp32); norms fp32.
"""

import os

import ml_dtypes
import numpy as np

import concourse.bacc as bacc
import concourse.mybir as mybir
import concourse.tile as tile
from concourse.bass import ts
from concourse.bass_utils import run_bass_kernel_spmd
from concourse.masks import make_identity

BF = ml_dtypes.bfloat16

NUM_USERS = 30000
NUM_ITEMS = 70000
N = 100000
D = 128
NL = 3
BATCH = 16384
NCORE = 8
SHARD = N // NCORE            # 12500
RB = 256                      # dest rows per block
NB = (SHARD + RB - 1) // RB   # 49 blocks/core (layers 1-2)
RS = 32768                    # range size (int16 index window)
NR = 4                        # ranges
GMAX = 32                     # max chunks per dma_gather call
SGC = 16                      # chunks per layer-1 stream tile
SGD = 8                       # chunks per layer-1 stream dma_start
BSH = BATCH // NCORE          # 2048
EPS2 = 1e-24
SLOPE = 0.01
SP = os.environ.get("KSP", "0") == "1"

f32 = mybir.dt.float32
f32r = mybir.dt.float32r
bf16 = mybir.dt.bfloat16
i16 = mybir.dt.int16

_cache = {}


def _key(meta):
    return repr(meta)


def _build(meta):
    (NCH0, chunk0, NCH1, groups1, chunk1, first_r1, last_r1,
     NCH2, groups2, chunk2, first_r2, last_r2, NS3, NB3, NF3T, F3R,
     fin_bounds, NFB) = meta
    nc = bacc.Bacc(num_devices=NCORE, num_swdge_queues=4)

    feat0 = nc.dram_tensor("feat0", [N, D], bf16, kind="ExternalInput")
    f0t = nc.dram_tensor("f0t", [D, SHARD], bf16, kind="ExternalInput")
    g0 = nc.dram_tensor("g0", [128, NCH0 * 128], bf16, kind="ExternalInput")
    erow0 = nc.dram_tensor("erow0", [128, NCH0], f32, kind="ExternalInput")
    evals0 = nc.dram_tensor("evals0", [128, NCH0], f32, kind="ExternalInput")
    eidx1 = nc.dram_tensor("eidx1", [128, NCH1 * 8], i16, kind="ExternalInput")
    erow1 = nc.dram_tensor("erow1", [128, NCH1], f32, kind="ExternalInput")
    evals1 = nc.dram_tensor("evals1", [128, NCH1], f32, kind="ExternalInput")
    eidx2 = nc.dram_tensor("eidx2", [128, NCH2 * 8], i16, kind="ExternalInput")
    erow2 = nc.dram_tensor("erow2", [128, NCH2], f32, kind="ExternalInput")
    evals2 = nc.dram_tensor("evals2", [128, NCH2], f32, kind="ExternalInput")
    idxf3 = nc.dram_tensor("idxf3", [128, NF3T // 16], i16,
                           kind="ExternalInput")
    wlin = nc.dram_tensor("wlin", [D, NL * D], bf16, kind="ExternalInput")
    wint = nc.dram_tensor("wint", [D, NL * D], bf16, kind="ExternalInput")
    biasc = nc.dram_tensor("biasc", [D, NL], f32, kind="ExternalInput")
    uidx = nc.dram_tensor("uidx", [128, NFB * 8], i16, kind="ExternalInput")
    iidx = nc.dram_tensor("iidx", [128, NFB * 8], i16, kind="ExternalInput")
    uidx3 = nc.dram_tensor("uidx3", [128, NFB * 8], i16, kind="ExternalInput")
    iidx3 = nc.dram_tensor("iidx3", [128, NFB * 8], i16, kind="ExternalInput")
    score = nc.dram_tensor("score", [128, NFB], f32, kind="ExternalOutput")

    add = mybir.AluOpType.add
    mult = mybir.AluOpType.mult
    is_equal = mybir.AluOpType.is_equal
    AF = mybir.ActivationFunctionType

    with tile.TileContext(nc) as tc:
        with (
            tc.tile_pool(name="const", bufs=1) as cp,
            tc.tile_pool(name="ft", bufs=1) as ftp,
            tc.tile_pool(name="st", bufs=2) as stp,
            tc.tile_pool(name="g", bufs=3) as gp,
            tc.tile_pool(name="fin", bufs=1) as fp_,
            tc.tile_pool(name="ot", bufs=8) as otp,
            tc.tile_pool(name="sb", bufs=3) as sbp,
            tc.tile_pool(name="plx", bufs=3, space="PSUM") as plx,
            tc.tile_pool(name="py", bufs=3, space="PSUM") as pyp,
            tc.tile_pool(name="ptp", bufs=2, space="PSUM") as ptp,
            tc.tile_pool(name="dram", bufs=1, space="DRAM") as dp,
        ):
            # ---------- constants ----------
            iota = cp.tile([128, RB], bf16)
            nc.gpsimd.iota(iota[:], pattern=[[1, RB]], base=0,
                           channel_multiplier=0,
                           allow_small_or_imprecise_dtypes=True)
            ident = cp.tile([128, 128], bf16)
            make_identity(nc, ident[:])
            epsb = cp.tile([128, 1], f32)
            nc.vector.memset(epsb[:], EPS2)

            wlin_b = cp.tile([128, NL * 128], bf16)
            nc.sync.dma_start(out=wlin_b[:], in_=wlin[:])
            wint_b = cp.tile([128, NL * 128], bf16)
            nc.sync.dma_start(out=wint_b[:], in_=wint[:])
            bias_sb = cp.tile([128, NL], f32)
            nc.sync.dma_start(out=bias_sb[:], in_=biasc[:])

            erow0_sb = cp.tile([128, NCH0], f32)
            nc.sync.dma_start(out=erow0_sb[:], in_=erow0[:])
            evals0_sb = cp.tile([128, NCH0], f32)
            nc.sync.dma_start(out=evals0_sb[:], in_=evals0[:])
            eidx1_sb = cp.tile([128, NCH1 * 8], i16)
            nc.sync.dma_start(out=eidx1_sb[:], in_=eidx1[:])
            erow1_sb = cp.tile([128, NCH1], f32)
            nc.sync.dma_start(out=erow1_sb[:], in_=erow1[:])
            evals1_sb = cp.tile([128, NCH1], f32)
            nc.sync.dma_start(out=evals1_sb[:], in_=evals1[:])
            eidx2_sb = cp.tile([128, NCH2 * 8], i16)
            nc.sync.dma_start(out=eidx2_sb[:], in_=eidx2[:])
            erow2_sb = cp.tile([128, NCH2], f32)
            nc.sync.dma_start(out=erow2_sb[:], in_=erow2[:])
            evals2_sb = cp.tile([128, NCH2], f32)
            nc.sync.dma_start(out=evals2_sb[:], in_=evals2[:])
            idxf3_sb = cp.tile([128, NF3T // 16], i16)
            nc.sync.dma_start(out=idxf3_sb[:], in_=idxf3[:])
            uidx_sb = cp.tile([128, NFB * 8], i16)
            nc.sync.dma_start(out=uidx_sb[:], in_=uidx[:])
            iidx_sb = cp.tile([128, NFB * 8], i16)
            nc.sync.dma_start(out=iidx_sb[:], in_=iidx[:])
            uidx3_sb = cp.tile([128, NFB * 8], i16)
            nc.sync.dma_start(out=uidx3_sb[:], in_=uidx3[:])
            iidx3_sb = cp.tile([128, NFB * 8], i16)
            nc.sync.dma_start(out=iidx3_sb[:], in_=iidx3[:])

            fta = ftp.tile([128, NB * RB], bf16, tag="fta")
            ftb = ftp.tile([128, NB * RB], bf16, tag="ftb")
            ft3in = ftp.tile([128, NB3 * RB], bf16, tag="ft3in")
            ft3out = ftp.tile([128, NB3 * RB], bf16, tag="ft3out")
            nc.vector.memset(fta[:, SHARD:], 0.0)
            nc.vector.memset(ft3out[:], 0.0)
            nc.vector.memset(ft3in[:], 0.0)
            nc.sync.dma_start(out=fta[:, :SHARD], in_=f0t[:])

            fshard = dp.tile([NB * RB, D], bf16)
            fshard3 = dp.tile([NS3, D], bf16)
            ags = [dp.tile([N, D], bf16, name=f"ag{i}", tag=f"ag{i}",
                           addr_space="Shared") for i in range(2)]
            ag3 = dp.tile([NCORE * NS3, D], bf16, name="ag3", tag="ag3",
                          addr_space="Shared")

            acc = cp.tile([128, NFB], f32)

            # ---------- helpers ----------
            def onehot(q, qrow, qval, tag):
                o = otp.tile([128, RB], bf16, tag="o", name=f"o_{tag}_{q}")
                nc.vector.tensor_scalar(
                    out=o[:], in0=iota[:],
                    scalar1=qrow[:, q:q + 1],
                    scalar2=qval[:, q:q + 1],
                    op0=is_equal, op1=mult,
                )
                return o

            def pass1(l, b, ftin, ftout):
                lxs = ftout[:, ts(b, RB)]
                fin_ = ftin[:, ts(b, RB)]
                pre1 = sbp.tile([128, RB], bf16, tag="pre1")
                nc.vector.tensor_tensor(out=pre1[:], in0=lxs, in1=fin_,
                                        op=add)
                pre2 = sbp.tile([128, RB], bf16, tag="pre2")
                nc.vector.tensor_tensor(out=pre2[:], in0=lxs, in1=fin_,
                                        op=mult)
                y = pyp.tile([128, RB], f32, tag="y")
                nc.tensor.matmul(out=y[:], lhsT=wlin_b[:, ts(l, 128)],
                                 rhs=pre1[:], start=True, stop=False)
                nc.tensor.matmul(out=y[:], lhsT=wint_b[:, ts(l, 128)],
                                 rhs=pre2[:], start=False, stop=True)
                nc.scalar.activation(out=lxs, in_=y[:], func=AF.Lrelu,
                                     bias=bias_sb[:, l:l + 1], scale=1.0,
                                     alpha=SLOPE)

            def pass2(l, nb, nrows, ftout, fsh):
                # normalize in normal layout: transpose, per-row norm,
                # fused scale into the shard-write copy
                for b in range(nb):
                    for h in range(2):
                        r0 = b * RB + h * 128
                        nr = min(128, nrows - r0)
                        if nr <= 0:
                            break
                        tp = ptp.tile([128, 128], bf16, tag="tp",
                                      name=f"tp{l}_{b}_{h}")
                        nc.tensor.transpose(
                            out=tp[:], in_=ftout[:, r0:r0 + 128],
                            identity=ident[:])
                        tpc = sbp.tile([128, 128], bf16, tag="tpc")
                        nc.vector.tensor_copy(tpc[:], tp[:])
                        sq2 = sbp.tile([128, 128], f32, tag="sq2")
                        nc.vector.tensor_tensor(out=sq2[:], in0=tpc[:],
                                                in1=tpc[:], op=mult)
                        ssum = sbp.tile([128, 1], f32, tag="ssum")
                        nc.vector.tensor_reduce(
                            out=ssum[:], in_=sq2[:],
                            axis=mybir.AxisListType.X, op=add)
                        rt2 = sbp.tile([128, 1], f32, tag="rt2")
                        nc.scalar.activation(out=rt2[:], in_=ssum[:],
                                             func=AF.Sqrt, bias=epsb[:])
                        inv1 = sbp.tile([128, 1], f32, tag="inv1")
                        nc.vector.reciprocal(inv1[:], rt2[:])
                        cpo = sbp.tile([128, 128], bf16, tag="cpo")
                        nc.vector.tensor_scalar(
                            out=cpo[:], in0=tpc[:], scalar1=inv1[:],
                            scalar2=None, op0=mult)
                        nc.sync.dma_start(out=fsh[r0:r0 + nr, :],
                                          in_=cpo[:nr, :])

            def final_level(li, srcf, u_sb, i_sb, bounds):
                ug = fp_.tile([128, NFB * 128], bf16, tag="ug",
                              name=f"ug{li}")
                nc.gpsimd.dma_gather(
                    ug[:].rearrange("p (c d) -> p c d", d=128),
                    srcf[:],
                    u_sb[:],
                    NFB * 128, NFB * 128, 128,
                    single_packet=SP,
                    queue_num=3,
                )
                ig = fp_.tile([128, NFB * 128], bf16, tag="ig",
                              name=f"ig{li}")
                for qi, (rr, c0, cn) in enumerate(bounds):
                    nc.gpsimd.dma_gather(
                        ig[:, c0 * 128:(c0 + cn) * 128].rearrange(
                            "p (c d) -> p c d", d=128),
                        srcf[rr * RS:, :],
                        i_sb[:, c0 * 8:(c0 + cn) * 8],
                        cn * 128, cn * 128, 128,
                        single_packet=SP,
                        queue_num=qi % 4,
                    )
                nc.vector.tensor_tensor(out=ug[:], in0=ug[:], in1=ig[:],
                                        op=mult)
                sc = sbp.tile([128, NFB], f32, tag="sc")
                nc.vector.tensor_reduce(
                    out=sc[:],
                    in_=ug[:].rearrange("p (c d) -> p c d", d=128),
                    axis=mybir.AxisListType.X, op=add)
                if li == 0:
                    nc.vector.tensor_copy(acc[:], sc[:])
                else:
                    nc.vector.tensor_tensor(out=acc[:], in0=acc[:],
                                            in1=sc[:], op=add)

            # ---------- level-0 final gather (overlaps layer-1 sweep) ----
            final_level(0, feat0, uidx_sb, iidx_sb, fin_bounds)

            # ---------- layer 1: streamed sources, block-major ----------
            cur_lx = None
            nst = (NCH0 + SGC - 1) // SGC
            for si in range(nst):
                q0 = si * SGC
                gn = min(SGC, NCH0 - q0)
                st = stp.tile([128, SGC * 128], bf16, tag="st",
                              name=f"st{si}")
                for j in range(0, gn, SGD):
                    jn = min(SGD, gn - j)
                    nc.sync.dma_start(
                        out=st[:, j * 128:(j + jn) * 128],
                        in_=g0[:, (q0 + j) * 128:(q0 + j + jn) * 128])
                for q in range(q0, q0 + gn):
                    b_q, is_first, is_last = chunk0[q]
                    o = onehot(q, erow0_sb, evals0_sb, "l0")
                    if is_first:
                        cur_lx = plx.tile([128, RB], f32, tag="lx",
                                          name=f"lx0_{q}")
                    nc.tensor.matmul(
                        out=cur_lx[:],
                        lhsT=st[:, ts(q - q0, 128)],
                        rhs=o[:],
                        start=is_first, stop=is_last,
                    )
                    if is_last:
                        nc.vector.tensor_copy(ftb[:, ts(b_q, RB)],
                                              cur_lx[:])
                        pass1(0, b_q, fta, ftb)
            pass2(0, NB, NB * RB, ftb, fshard)
            # reload normalized features, transposed, as layer-2 dense input
            nc.sync.dma_start_transpose(out=fta[:], in_=fshard[:])
            nc.gpsimd.collective_compute(
                "AllGather", mybir.AluOpType.bypass,
                replica_groups=[list(range(NCORE))],
                ins=[fshard[:SHARD, :].opt()], outs=[ags[0].opt()],
            )
            final_level(1, ags[0], uidx_sb, iidx_sb, fin_bounds)

            # ---------- layers 2-3: gathered sources, range-major --------
            for l, (NCHl, groups, chunk_info, first_r, last_r, eidx_sb,
                    erow_sb, evals_sb, src, ftin, ftout, nb, nrows, fsh,
                    agout) in enumerate((
                (NCH1, groups1, chunk1, first_r1, last_r1, eidx1_sb,
                 erow1_sb, evals1_sb, ags[0], fta, ftb, NB, NB * RB,
                 fshard, ags[1]),
                (NCH2, groups2, chunk2, first_r2, last_r2, eidx2_sb,
                 erow2_sb, evals2_sb, ags[1], ft3in, ft3out, NB3, NS3,
                 fshard3, ag3),
            ), start=1):
                cur_lx = None
                for gi, (gr, q0, gn) in enumerate(groups):
                    gt = gp.tile([128, GMAX * 128], bf16, tag="g",
                                 name=f"g{l}_{gi}")
                    nc.gpsimd.dma_gather(
                        gt[:, :gn * 128].rearrange("p (c d) -> p c d",
                                                   d=128),
                        src[gr * RS:, :],
                        eidx_sb[:, q0 * 8:(q0 + gn) * 8],
                        gn * 128, gn * 128, 128,
                        single_packet=SP,
                        queue_num=gi % 4,
                    )
                    for q in range(q0, q0 + gn):
                        r_q, b_q, is_first, is_last, loc = chunk_info[q]
                        o = onehot(q, erow_sb, evals_sb, f"l{l}")
                        if is_first:
                            cur_lx = plx.tile([128, RB], f32, tag="lx",
                                              name=f"lx{l}_{q}")
                        nc.tensor.matmul(
                            out=cur_lx[:],
                            lhsT=gt[:, ts(loc, 128)],
                            rhs=o[:],
                            start=is_first, stop=is_last,
                        )
                        if is_last:
                            dst = ftout[:, ts(b_q, RB)]
                            if first_r[b_q] == r_q:
                                nc.scalar.copy(dst, cur_lx[:])
                            else:
                                nc.vector.tensor_tensor(
                                    out=dst, in0=dst, in1=cur_lx[:],
                                    op=add)
                            if last_r[b_q] == r_q:
                                pass1(l, b_q, ftin, ftout)
                pass2(l, nb, nrows, ftout, fsh)
                nc.gpsimd.collective_compute(
                    "AllGather", mybir.AluOpType.bypass,
                    replica_groups=[list(range(NCORE))],
                    ins=[fsh[:SHARD, :].opt() if l == 1 else fsh.opt()],
                    outs=[agout.opt()],
                )
                if l == 1:
                    # layer-3 dense inputs: transposed gather of the compact
                    # rows' layer-2 features (per int16 source range)
                    rb0 = 0
                    for r in range(NR):
                        fn = F3R[r]
                        if fn == 0:
                            continue
                        nc.gpsimd.dma_gather(
                            ft3in[:, rb0:rb0 + fn].rearrange(
                                "p (c d) -> p c d", c=1),
                            ags[1][r * RS:, :],
                            idxf3_sb[:, rb0 // 16:(rb0 + fn) // 16],
                            fn, fn, 128,
                            transpose=True,
                            single_packet=SP,
                            queue_num=r % 4,
                        )
                        rb0 += fn
                    final_level(2, ags[1], uidx_sb, iidx_sb, fin_bounds)
                else:
                    final_level(3, ag3, uidx3_sb, iidx3_sb,
                                [(0, 0, NFB)])
            nc.sync.dma_start(out=score[:], in_=acc[:])

    nc.compile()
    return nc


def _chunks_of(counts, kk):
    """Block-major chunk table: [(b, is_first, is_last)] and base offsets."""
    info = []
    base = {}
    q = 0
    for b, k in enumerate(kk):
        if k == 0:
            continue
        base[b] = q
        for j in range(k):
            info.append((b, j == 0, j == k - 1))
        q += k
    return info, base, q


def _pack_spmm_ranged(rows_l, cols_l, vals_l, nshard, nblocks):
    """Bucket edges by (core, range, block) with shared chunk counts.

    rows_l are destination indices in the (possibly compact) shard space
    (core = row // nshard); cols_l index the source table (range split).
    Returns per-core eidx/erow/evals arrays plus chunk/group metadata.
    """
    core = rows_l // nshard
    local = rows_l - core * nshard
    blk = local // RB
    rowl = (local - blk * RB).astype(np.float32)
    rng = cols_l // RS
    col_local = (cols_l - rng * RS).astype(np.int16)

    nbk = nblocks
    bkey = ((core * NR + rng) * nbk + blk).astype(np.int64)
    order = np.argsort(bkey, kind="stable")
    bkey_s = bkey[order]
    counts = np.bincount(bkey_s, minlength=NCORE * NR * nbk)
    counts = counts.reshape(NCORE, NR, nbk)
    k2 = np.ceil(counts.max(axis=0) / 128).astype(np.int64)  # [NR, nbk]

    chunk_base = np.zeros((NR, nbk), dtype=np.int64)
    nch = 0
    chunk_info = []          # (r, b, is_first, is_last, loc)
    groups = []              # (r, q0, n)
    first_r = [None] * nbk
    last_r = [None] * nbk
    for r in range(NR):
        for b in range(nbk):
            k = int(k2[r, b])
            if k == 0:
                continue
            if first_r[b] is None:
                first_r[b] = r
            last_r[b] = r
            chunk_base[r, b] = nch
            for j in range(k):
                chunk_info.append([r, b, j == 0, j == k - 1, 0])
            nch += k
    q = 0
    while q < nch:
        r = chunk_info[q][0]
        n = 1
        while (q + n < nch and n < GMAX and chunk_info[q + n][0] == r):
            n += 1
        groups.append((r, q, n))
        for j in range(n):
            chunk_info[q + j][4] = j
        q += n
    chunk_info = tuple(tuple(x) for x in chunk_info)

    starts = np.zeros(NCORE * NR * nbk, dtype=np.int64)
    np.cumsum(counts.reshape(-1)[:-1], out=starts[1:])
    pos = np.arange(len(bkey_s), dtype=np.int64) - starts[bkey_s]
    core_s = core[order]
    q_of_edge = chunk_base[rng[order], blk[order]] + pos // 128
    p_of_edge = pos % 128

    eidx_arr = np.zeros((NCORE, 16, nch * 8), dtype=np.int16)
    erow_arr = np.zeros((NCORE, 128, nch), dtype=np.float32)
    eval_arr = np.zeros((NCORE, 128, nch), dtype=np.float32)
    eidx_arr[core_s, p_of_edge % 16, q_of_edge * 8 + p_of_edge // 16] = \
        col_local[order]
    erow_arr[core_s, p_of_edge, q_of_edge] = rowl[order]
    eval_arr[core_s, p_of_edge, q_of_edge] = vals_l[order]

    return (nch, tuple(groups), chunk_info,
            tuple(0 if x is None else x for x in first_r),
            tuple(0 if x is None else x for x in last_r),
            eidx_arr, erow_arr, eval_arr)


def _pack_inputs(userIdx, itemIdx, rows, cols, vals, uEmbd, iEmbd,
                 Wlin, blin, Wint, bint):
    rows = np.asarray(rows, dtype=np.int64)
    cols = np.asarray(cols, dtype=np.int64)
    vals = np.asarray(vals, dtype=np.float32)
    userIdx = np.asarray(userIdx, dtype=np.int64)
    itemIdx = np.asarray(itemIdx, dtype=np.int64)

    feat0 = np.ascontiguousarray(
        np.concatenate([np.asarray(uEmbd, np.float32),
                        np.asarray(iEmbd, np.float32)], axis=0).astype(BF))

    # ---- layer 1: host-pregathered stream, block-major by dest ----
    core0 = rows // SHARD
    local0 = rows - core0 * SHARD
    blk0 = local0 // RB
    rowl0 = (local0 - blk0 * RB).astype(np.float32)
    bkey0 = (core0 * NB + blk0).astype(np.int64)
    order0 = np.argsort(bkey0, kind="stable")
    counts0 = np.bincount(bkey0[order0], minlength=NCORE * NB)
    counts0 = counts0.reshape(NCORE, NB)
    k0 = np.ceil(counts0.max(axis=0) / 128).astype(np.int64)
    chunk0_l, base0, NCH0 = _chunks_of(counts0.max(axis=0), k0)
    base0_arr = np.zeros(NB, dtype=np.int64)
    for b, q in base0.items():
        base0_arr[b] = q
    starts0 = np.zeros(NCORE * NB, dtype=np.int64)
    np.cumsum(counts0.reshape(-1)[:-1], out=starts0[1:])
    pos0 = np.arange(len(order0), dtype=np.int64) - starts0[bkey0[order0]]
    core0_s = core0[order0]
    q0_of = base0_arr[blk0[order0]] + pos0 // 128
    p0_of = pos0 % 128

    col_of_slot = np.full((NCORE, 128, NCH0), -1, dtype=np.int64)
    col_of_slot[core0_s, p0_of, q0_of] = cols[order0]
    erow0_arr = np.zeros((NCORE, 128, NCH0), dtype=np.float32)
    eval0_arr = np.zeros((NCORE, 128, NCH0), dtype=np.float32)
    erow0_arr[core0_s, p0_of, q0_of] = rowl0[order0]
    eval0_arr[core0_s, p0_of, q0_of] = vals[order0]
    feat0z = np.concatenate([feat0, np.zeros((1, D), BF)], axis=0)
    # g0[c][p, q*128:(q+1)*128] = feat0[col_of_slot[c, p, q]]
    g0_arr = feat0z[col_of_slot]            # [NCORE, 128, NCH0, D]
    g0_arr = np.ascontiguousarray(
        g0_arr.reshape(NCORE, 128, NCH0 * D))

    # ---- layer 2: full shard, ranged gather from ags[0] ----
    (NCH1, groups1, chunk1, first_r1, last_r1,
     eidx1_arr, erow1_arr, eval1_arr) = _pack_spmm_ranged(
        rows, cols, vals, SHARD, NB)

    # ---- layer 3: pruned to referenced rows, re-sharded ----
    irow = itemIdx + NUM_USERS
    refset = np.union1d(userIdx, irow)       # sorted unique, ascending
    # per-core compact shard: contiguous split of the sorted referenced set
    nref = len(refset)
    # round-robin so each core's rows spread evenly over the int16 ranges
    core_of_ref = np.arange(nref) % NCORE
    # compute per-core per-range counts
    rng_ref = refset // RS
    cnt_cr = np.zeros((NCORE, NR), dtype=np.int64)
    for c in range(NCORE):
        cnt_cr[c] = np.bincount(rng_ref[core_of_ref == c], minlength=NR)
    F3R = (np.ceil(cnt_cr.max(axis=0) / 128) * 128).astype(np.int64)
    NF3T = int(F3R.sum())
    NS3 = NF3T
    NB3 = (NS3 + RB - 1) // RB
    rbase = np.zeros(NR, dtype=np.int64)
    np.cumsum(F3R[:-1], out=rbase[1:])
    # local position of each referenced row
    localpos = np.zeros(nref, dtype=np.int64)
    idxf3_arr = np.zeros((NCORE, 16, NF3T // 16), dtype=np.int16)
    for c in range(NCORE):
        m = core_of_ref == c
        rs = refset[m]
        rr = rng_ref[m]
        lp = np.zeros(len(rs), dtype=np.int64)
        for r in range(NR):
            mr = rr == r
            lp[mr] = rbase[r] + np.arange(int(mr.sum()))
        localpos[m] = lp
        # transposed-gather indices for layer-3 dense inputs (per range)
        jj = np.zeros(NF3T, dtype=np.int16)
        for r in range(NR):
            jr = np.zeros(F3R[r], dtype=np.int16)
            jr[:int(cnt_cr[c][r])] = (rs[rr == r] - r * RS).astype(np.int16)
            jj[rbase[r]:rbase[r] + F3R[r]] = jr
        idxf3_arr[c, np.arange(NF3T) % 16,
                  (np.arange(NF3T) // 128) * 8 +
                  (np.arange(NF3T) % 128) // 16] = jj
    compact = core_of_ref * NS3 + localpos   # global compact id per ref row
    cmap = np.full(N, -1, dtype=np.int64)
    cmap[refset] = compact

    keep = cmap[rows] >= 0
    (NCH2, groups2, chunk2, first_r2, last_r2,
     eidx2_arr, erow2_arr, eval2_arr) = _pack_spmm_ranged(
        cmap[rows[keep]], cols[keep], vals[keep], NS3, NB3)
    # note: transposed-gather ranges differ per range of SOURCE cols for
    # idxf3 (handled above); layer-3 gathers read full ags[1] (4 ranges).

    # ---- weights ----
    wlin_h = np.ascontiguousarray(
        np.asarray(Wlin, np.float32).transpose(1, 0, 2).reshape(D, NL * D)
        .astype(BF))
    wint_h = np.ascontiguousarray(
        np.asarray(Wint, np.float32).transpose(1, 0, 2).reshape(D, NL * D)
        .astype(BF))
    biasc = np.ascontiguousarray(
        (np.asarray(blin, np.float32) + np.asarray(bint, np.float32)).T)

    # ---- final stage: bucket item rows by range (levels 0-2) ----
    ir = irow // RS
    nfb_counts = np.zeros((NCORE, NR), dtype=np.int64)
    perms = []
    for c in range(NCORE):
        sl = slice(c * BSH, (c + 1) * BSH)
        o = np.argsort(ir[sl], kind="stable")
        perms.append(o)
        nfb_counts[c] = np.bincount(ir[sl][o], minlength=NR)
    bucket_chunks = np.ceil(nfb_counts.max(axis=0) / 128).astype(np.int64)
    fin_bounds = []
    c0 = 0
    for r in range(NR):
        n = int(bucket_chunks[r])
        if n == 0:
            continue
        fin_bounds.append((r, c0, n))
        c0 += n
    NFB = c0

    uidx_arr = np.zeros((NCORE, 16, NFB * 8), dtype=np.int16)
    iidx_arr = np.zeros((NCORE, 16, NFB * 8), dtype=np.int16)
    uidx3_arr = np.zeros((NCORE, 16, NFB * 8), dtype=np.int16)
    iidx3_arr = np.zeros((NCORE, 16, NFB * 8), dtype=np.int16)
    inv_perm = np.full((NCORE, NFB * 128), -1, dtype=np.int64)
    for c in range(NCORE):
        sl = slice(c * BSH, (c + 1) * BSH)
        o = perms[c]
        u_s = userIdx[sl][o]
        i_s = irow[sl][o]
        r_s = ir[sl][o]
        jpos = np.zeros(BSH, dtype=np.int64)
        for (r, b0, nchk) in fin_bounds:
            m = r_s == r
            jpos[m] = b0 * 128 + np.arange(int(m.sum()))
        jp16 = jpos % 16
        jcol = (jpos // 128) * 8 + (jpos % 128) // 16
        uidx_arr[c, jp16, jcol] = u_s.astype(np.int16)
        iidx_arr[c, jp16, jcol] = (i_s - r_s * RS).astype(np.int16)
        uidx3_arr[c, jp16, jcol] = cmap[u_s].astype(np.int16)
        iidx3_arr[c, jp16, jcol] = cmap[i_s].astype(np.int16)
        inv_perm[c, jpos] = np.arange(c * BSH, (c + 1) * BSH)[o]

    meta = (NCH0, tuple(chunk0_l), NCH1, groups1, chunk1, first_r1,
            last_r1, NCH2, groups2, chunk2, first_r2, last_r2, NS3, NB3,
            NF3T, tuple(int(x) for x in F3R), tuple(fin_bounds), NFB)

    in_maps = []
    for c in range(NCORE):
        f0t = np.ascontiguousarray(feat0[c * SHARD:(c + 1) * SHARD].T)
        in_maps.append({
            "feat0": feat0,
            "f0t": f0t,
            "g0": g0_arr[c],
            "erow0": np.ascontiguousarray(erow0_arr[c]),
            "evals0": np.ascontiguousarray(eval0_arr[c]),
            "eidx1": np.ascontiguousarray(np.tile(eidx1_arr[c], (8, 1))),
            "erow1": np.ascontiguousarray(erow1_arr[c]),
            "evals1": np.ascontiguousarray(eval1_arr[c]),
            "eidx2": np.ascontiguousarray(np.tile(eidx2_arr[c], (8, 1))),
            "erow2": np.ascontiguousarray(erow2_arr[c]),
            "evals2": np.ascontiguousarray(eval2_arr[c]),
            "idxf3": np.ascontiguousarray(np.tile(idxf3_arr[c], (8, 1))),
            "wlin": wlin_h,
            "wint": wint_h,
            "biasc": biasc,
            "uidx": np.ascontiguousarray(np.tile(uidx_arr[c], (8, 1))),
            "iidx": np.ascontiguousarray(np.tile(iidx_arr[c], (8, 1))),
            "uidx3": np.ascontiguousarray(np.tile(uidx3_arr[c], (8, 1))),
            "iidx3": np.ascontiguousarray(np.tile(iidx3_arr[c], (8, 1))),
        })
    return meta, in_maps, inv_perm


def kernel(**inputs) -> np.ndarray:
    meta, in_maps, inv_perm = _pack_inputs(**inputs)
    key = _key(meta)
    if key not in _cache:
        _cache[key] = _build(meta)
    nc = _cache[key]
    res = run_bass_kernel_spmd(nc, in_maps, list(range(NCORE)))
    out = np.empty(BATCH, dtype=np.float32)
    NFB = meta[17]
    for c in range(NCORE):
        sc = res.results[c]["score"]  # [128, NFB]
        vals_j = sc[np.arange(NFB * 128) % 128, np.arange(NFB * 128) // 128]
        valid = inv_perm[c] >= 0
        out[inv_perm[c][valid]] = vals_j[valid]
    return out
